# revision 1
# baseline (speedup 1.0000x reference)
"""MoNet (GMM graph conv) on Trainium2 — 8-core SPMD Bass/Tile kernel.

Sharding: dst-node slices per core (edge-parallel within core), with node
relabeling into per-core "slot space". Per core, uniform SPMD program:

 - window = 32 dst slots, 5 edge-tiles of 128 slots (3 "even-class" +
   2 "odd-class"); class = src-node table-row parity. Two stride-2 table
   views keep dma_gather's int16 indices in range (26624 rows each).
 - per layer: dma_gather h rows (bf16 256B rows: 64 feats + ones col) ->
   per-tile PE matmul (lhsT = gathered [128,65], rhs = S3' = host 0/1
   one-hot x on-device gauss, built by DVE) accumulating u^T [65,96] per
   window in PSUM -> dense fc matmuls (f32) -> BN via ones-matmul stats +
   AllReduce -> relu (+residual) -> bf16 staging -> AllGather into table.
"""
import sys, os
import numpy as np

if "/opt/trn_rl_repo" not in sys.path:
    sys.path.insert(0, "/opt/trn_rl_repo")

import ml_dtypes
from concourse import bass, bacc, mybir, tile
from concourse import bass_utils
from concourse.masks import make_identity

AluOp = mybir.AluOpType
Act = mybir.ActivationFunctionType
F32 = mybir.dt.float32
BF16 = mybir.dt.bfloat16
I16 = mybir.dt.int16
U16 = mybir.dt.uint16

NCORES = 8
EPS = 1e-5
T_EV, T_OD = 3, 2
TPW = T_EV + T_OD

GEOM_REAL = dict(n=50000, e=800000, in_dim=128, hid=64, k=3, pdim=2,
                 ncls=16, nhl=3, W=208, wpchunk=16)


def derive(geom):
    g = dict(geom)
    g["npc"] = g["W"] * 32                # dst slots per core
    g["NG"] = g["npc"] // 128             # 128-slot groups per core
    g["TPC"] = g["W"] * TPW               # edge tiles per core
    g["NCH"] = g["W"] // g["wpchunk"]     # chunks per layer
    g["TCH"] = g["wpchunk"] * TPW         # tiles per chunk
    g["n_rows"] = NCORES * g["npc"]       # table rows
    g["EVN"] = g["wpchunk"] * T_EV * 128  # idxs per even gather call
    g["ODN"] = g["wpchunk"] * T_OD * 128
    assert g["n_rows"] // 2 <= 32767
    return g


# ---------------------------------------------------------------------------
# host preprocessing (pure integer/index manipulation)
# ---------------------------------------------------------------------------

def preprocess(edge_index, geom):
    g = derive(geom)
    n, W, npc = g["n"], g["W"], g["npc"]
    row = np.asarray(edge_index[0], np.int64)
    col = np.asarray(edge_index[1], np.int64)
    deg_r = np.bincount(row, minlength=n).astype(np.int64)
    deg_c = np.bincount(col, minlength=n).astype(np.int64)

    # 1) nodes -> cores (snake deal by in-degree for balanced edge counts)
    order = np.argsort(-deg_c, kind="stable")
    core_of = np.empty(n, np.int64)
    blk = np.arange(n) // NCORES
    pos = np.arange(n) % NCORES
    snake = np.where(blk % 2 == 0, pos, NCORES - 1 - pos)
    core_of[order] = snake

    # 2) class A (even rows) = per-core top half by out-degree
    is_a = np.zeros(n, bool)
    for c in range(NCORES):
        nds = np.flatnonzero(core_of == c)
        half = min((len(nds) + 1) // 2, W * 16)
        topa = nds[np.argsort(-deg_r[nds], kind="stable")][:half]
        is_a[topa] = True

    src_a = is_a[row]
    in_ev = np.bincount(col[src_a], minlength=n).astype(np.int64)
    in_od = np.bincount(col[~src_a], minlength=n).astype(np.int64)

    # 3) per-core window packing (first-fit decreasing)
    cap_ev, cap_od = T_EV * 128, T_OD * 128
    slot_of = np.full(n, -1, np.int64)
    for c in range(NCORES):
        nds = np.flatnonzero(core_of == c)
        nds = nds[np.argsort(-(in_ev[nds] + in_od[nds]), kind="stable")]
        wev = np.zeros(W, np.int64); wod = np.zeros(W, np.int64)
        wna = np.zeros(W, np.int64); wnb = np.zeros(W, np.int64)
        for nd in nds:
            a = bool(is_a[nd])
            for w in range(W):
                if a and wna[w] >= 16: continue
                if (not a) and wnb[w] >= 16: continue
                if wev[w] + in_ev[nd] > cap_ev: continue
                if wod[w] + in_od[nd] > cap_od: continue
                if a:
                    j = 2 * wna[w]; wna[w] += 1
                else:
                    j = 2 * wnb[w] + 1; wnb[w] += 1
                wev[w] += in_ev[nd]; wod[w] += in_od[nd]
                slot_of[nd] = c * npc + w * 32 + j
                break
            else:
                raise RuntimeError(f"window packing failed (core {c})")

    assert (slot_of >= 0).all()
    # class A nodes landed on even global rows
    assert (slot_of[is_a] % 2 == 0).all() and (slot_of[~is_a] % 2 == 1).all()

    g.update(core_of=core_of, slot_of=slot_of, deg_r=deg_r, deg_c=deg_c)

    # 4) per-core edge-slot layouts
    NCH, TCH, TPC = g["NCH"], g["TCH"], g["TPC"]
    wpc = g["wpchunk"]
    e_core = core_of[col]
    e_slot = slot_of[col] % npc
    e_w = e_slot // 32
    e_j = e_slot % 32
    e_view = (slot_of[row] // 2).astype(np.int64)   # stride-2 view index

    per_core = []
    for c in range(NCORES):
        idx_ev = np.zeros((NCH, 128, g["EVN"] // 16), np.int16)
        idx_od = np.zeros((NCH, 128, g["ODN"] // 16), np.int16)
        eq = np.zeros((NCH, 128, 32, TCH), ml_dtypes.bfloat16)
        dr = np.zeros((128, TPC), np.float32)
        dc = np.zeros((128, TPC), np.float32)

        sel = np.flatnonzero(e_core == c)
        ew, ej, ecls = e_w[sel], e_j[sel], src_a[sel]
        evi = e_view[sel]
        edr = deg_r[row[sel]].astype(np.float32)
        edc = deg_c[col[sel]].astype(np.float32)
        # order edges by (window, class) once
        okey = ew * 2 + (~ecls).astype(np.int64)
        eorder = np.argsort(okey, kind="stable")
        bnd = np.searchsorted(okey[eorder], np.arange(2 * W + 1))
        for w in range(W):
            ch, wl = divmod(w, wpc)
            for a_cls in (True, False):
                kk = w * 2 + (0 if a_cls else 1)
                eids = eorder[bnd[kk]:bnd[kk + 1]]
                ne = len(eids)
                ntile = T_EV if a_cls else T_OD
                base_tti = 0 if a_cls else T_EV
                assert ne <= ntile * 128
                q = np.arange(ne)
                tti = base_tti + q // 128
                p = q % 128
                tg = wl * TPW + tti                      # tile within chunk
                mcall = (wl * ntile + (q // 128)) * 128 + p
                tgt = idx_ev if a_cls else idx_od
                tgt[ch][mcall % 16, mcall // 16] = evi[eids].astype(np.int16)
                eq[ch][p, ej[eids], tg] = 1.0
                dr[p, ch * TCH + tg] = edr[eids]
                dc[p, ch * TCH + tg] = edc[eids]
        idx_ev = np.tile(idx_ev[:, :16, :], (1, 8, 1))
        idx_od = np.tile(idx_od[:, :16, :], (1, 8, 1))
        per_core.append(dict(
            idx_ev=idx_ev, idx_od=idx_od,
            eq=np.ascontiguousarray(eq.reshape(NCH, 128, 32 * TCH)).view(np.uint16),
            dr=dr, dc=dc))
    g["per_core"] = per_core
    return g


# ---------------------------------------------------------------------------
# device program
# ---------------------------------------------------------------------------

def build(tc, outs, ins, g):
    nc = tc.nc
    W, npc, NG = g["W"], g["npc"], g["NG"]
    TPC, NCH, TCH, wpc = g["TPC"], g["NCH"], g["TCH"], g["wpchunk"]
    HID, KK, NCLS, NHL = g["hid"], g["k"], g["ncls"], g["nhl"]
    n_rows = g["n_rows"]
    nn = g["n"]

    import contextlib
    stack = contextlib.ExitStack()
    sbc = stack.enter_context(tc.tile_pool(name="sbc", bufs=1))
    sb1 = stack.enter_context(tc.tile_pool(name="sb1", bufs=1))
    sb = stack.enter_context(tc.tile_pool(name="sb", bufs=2))
    ps = stack.enter_context(tc.tile_pool(name="ps", bufs=8, space="PSUM"))
    dram = stack.enter_context(tc.tile_pool(name="dram", bufs=1, space="DRAM"))

    # ---- constants / persistent state
    onesrow = sbc.tile([1, 128], F32); nc.vector.memset(onesrow[:], 1.0)
    onescol = sbc.tile([128, 1], F32); nc.vector.memset(onescol[:], 1.0)
    ident = sbc.tile([HID, HID], F32)
    nc.sync.dma_start(out=ident[:], in_=ins["ident"][:])
    stage = sbc.tile([128, NG, 128], BF16)
    nc.vector.memset(stage[:], 0.0)
    nc.vector.memset(stage[:, :, 64:65], 1.0)
    srcs = sbc.tile([128, TPC], F32)
    dsts = sbc.tile([128, TPC], F32)
    gauss = sbc.tile([128, KK, TPC], BF16)

    table = dram.tile([n_rows, 128], BF16)
    stage_d = dram.tile([npc, 128], BF16)
    stats_in = dram.tile([HID, 2], F32)
    stats_out = dram.tile([HID, 2], F32)

    zz = sbc.tile([HID, 2], F32)
    nc.vector.memset(zz[:], 0.0)
    nc.sync.dma_start(out=stats_in[:], in_=zz[:])
    nc.sync.dma_start(out=stats_out[:], in_=zz[:])
    # init whole table from the zeroed stage tile (covers pad rows too)
    for c in range(NCORES):
        nc.sync.dma_start(
            out=table[c * npc:(c + 1) * npc, :]
                .rearrange("(gp p) c -> p gp c", p=128),
            in_=stage[:])

    tbl_ev = table[:].rearrange("(m two) c -> m (two c)", two=2)[:, 0:128]
    tbl_od = table[:].rearrange("(m two) c -> m (two c)", two=2)[:, 128:256]

    # ---- prologue: pseudo coords
    with tc.tile_pool(name="pro", bufs=1) as pro:
        drt = pro.tile([128, TPC], F32)
        nc.sync.dma_start(out=drt[:], in_=ins["dr"][:])
        dct = pro.tile([128, TPC], F32)
        nc.sync.dma_start(out=dct[:], in_=ins["dc"][:])
        t0 = pro.tile([128, TPC], F32)
        for dsrc, dout in ((drt, srcs), (dct, dsts)):
            nc.vector.tensor_scalar(t0[:], dsrc[:], 1.0, None, AluOp.add)
            nc.scalar.sqrt(t0[:], t0[:])
            nc.vector.reciprocal(dout[:], t0[:])

    NO_CC = os.environ.get("MONET_NO_CC", "0") == "1"
    NHID_RUN = int(os.environ.get("MONET_NLAYERS", str(NHL)))

    def push_table(h_flat):
        # h_flat [128, NG*64] f32 -> stage bf16 -> DRAM -> AllGather table
        nc.vector.tensor_copy(
            out=stage[:, :, 0:64],
            in_=h_flat.rearrange("p (g c) -> p g c", c=64))
        nc.sync.dma_start(
            out=stage_d[:].rearrange("(gp p) c -> p gp c", p=128),
            in_=stage[:])
        if NO_CC:
            nc.sync.dma_start(out=table[0:npc, :], in_=stage_d[:])
            return
        nc.gpsimd.collective_compute(
            "AllGather", AluOp.bypass, replica_groups=[list(range(NCORES))],
            ins=[stage_d[:].opt()], outs=[table[:].opt()])

    # ---- embed: h0 = featT.T @ emb_w + emb_b
    h_cur = sb.tile([128, NG * HID], F32, tag="h")
    with tc.tile_pool(name="emb", bufs=2) as emb:
        embw = emb.tile([128, HID], F32, tag="embw")
        nc.sync.dma_start(out=embw[:], in_=ins["emb_w"][:])
        ebrow = emb.tile([1, HID], F32, tag="ebrow")
        nc.sync.dma_start(out=ebrow[:], in_=ins["emb_b"][:])
        for gi in range(NG):
            ft = emb.tile([128, 128], F32, tag="ft")
            nc.sync.dma_start(out=ft[:], in_=ins["featT"][:, gi * 128:(gi + 1) * 128])
            ep = ps.tile([128, HID], F32, tag="ps")
            nc.tensor.matmul(out=ep[:], lhsT=ft[:],
                             rhs=embw[:], start=True, stop=True)
            nc.scalar.copy(out=h_cur[:, gi * HID:(gi + 1) * HID], in_=ep[:])
        ebp = ps.tile([128, HID], F32, tag="ps")
        nc.tensor.matmul(out=ebp[:], lhsT=onesrow[:], rhs=ebrow[:],
                         start=True, stop=True)
        ebrep = emb.tile([128, HID], F32)
        nc.scalar.copy(out=ebrep[:], in_=ebp[:])
        nc.vector.tensor_tensor(
            out=h_cur[:], in0=h_cur[:],
            in1=ebrep[:].rearrange("p (o c) -> p o c", o=1)
                .broadcast_to([128, NG, HID]),
            op=AluOp.add)
    push_table(h_cur[:])

    # ---- layers
    for li in list(range(NHID_RUN)) + [NHL]:
        last = li == NHL
        OUT = NCLS if last else HID

        # scalars row: [w00 w01 w10 w11 b0 b1 | mu k*2+d | isg k*2+d]
        scal_row = sb1.tile([1, 32], F32, tag="scalrow")
        nc.vector.memset(scal_row[:], 0.0)
        if last:
            nc.sync.dma_start(out=scal_row[:, 0:4], in_=ins["pp_w_l"][:])
            nc.sync.dma_start(out=scal_row[:, 4:6], in_=ins["pp_b_l"][:])
            nc.sync.dma_start(out=scal_row[:, 6:6 + 2 * KK], in_=ins["mu_l"][:])
            nc.sync.dma_start(out=scal_row[:, 18:18 + 2 * KK],
                              in_=ins["inv_sigma_l"][:])
        else:
            nc.sync.dma_start(out=scal_row[:, 0:4], in_=ins["pp_w"][li])
            nc.sync.dma_start(out=scal_row[:, 4:6], in_=ins["pp_b"][li])
            nc.sync.dma_start(out=scal_row[:, 6:6 + 2 * KK], in_=ins["mu"][li])
            nc.sync.dma_start(out=scal_row[:, 18:18 + 2 * KK],
                              in_=ins["inv_sigma"][li])
        scp = ps.tile([128, 32], F32, tag="ps")
        nc.tensor.matmul(out=scp[:], lhsT=onesrow[:], rhs=scal_row[:],
                         start=True, stop=True)
        scal = sb1.tile([128, 32], F32, tag="scal")
        nc.scalar.copy(out=scal[:], in_=scp[:])

        def sc(j):
            return scal[:, j:j + 1]

        # gauss[k] = exp(-0.5*(((ps0-mu_k0)*is_k0)^2 + ((ps1-mu_k1)*is_k1)^2))
        ps0 = sb1.tile([128, TPC], F32, tag="ps0")
        ps1 = sb1.tile([128, TPC], F32, tag="ps1")
        ta = sb1.tile([128, TPC], F32, tag="ta")
        tb = sb1.tile([128, TPC], F32, tag="tb")
        for (pst, wA, wB, bB) in ((ps0, 0, 2, 4), (ps1, 1, 3, 5)):
            nc.vector.tensor_scalar(ta[:], srcs[:], sc(wA), None, AluOp.mult)
            nc.vector.tensor_scalar(tb[:], dsts[:], sc(wB), None, AluOp.mult)
            nc.vector.tensor_tensor(out=ta[:], in0=ta[:], in1=tb[:], op=AluOp.add)
            nc.scalar.activation(pst[:], ta[:], Act.Tanh, bias=sc(bB), scale=1.0)
        for k in range(KK):
            nc.vector.tensor_scalar(ta[:], ps0[:], sc(6 + 2 * k), sc(18 + 2 * k),
                                    AluOp.subtract, AluOp.mult)
            nc.vector.tensor_scalar(tb[:], ps1[:], sc(7 + 2 * k), sc(19 + 2 * k),
                                    AluOp.subtract, AluOp.mult)
            nc.scalar.square(ta[:], ta[:])
            nc.scalar.square(tb[:], tb[:])
            nc.vector.tensor_tensor(out=ta[:], in0=ta[:], in1=tb[:], op=AluOp.add)
            nc.scalar.activation(gauss[:, k, :], ta[:], Act.Exp,
                                 bias=0.0, scale=-0.5)

        # dense weights [65, K*OUT]
        fcwb = sb1.tile([65, KK * OUT], F32, tag="fcwb")
        if last:
            nc.sync.dma_start(out=fcwb[0:64, :], in_=ins["fc_w_l"][:])
            nc.sync.dma_start(out=fcwb[64:65, :], in_=ins["fc_b_l"][:])
        else:
            nc.sync.dma_start(out=fcwb[0:64, :], in_=ins["fc_w"][li])
            nc.sync.dma_start(out=fcwb[64:65, :], in_=ins["fc_b"][li])

        agg = sb1.tile([128, NG * OUT], F32, tag="aggsb")

        # ---- edge pipeline
        for ch in range(NCH):
            iev = sb.tile([128, g["EVN"] // 16], I16, tag="iev")
            nc.sync.dma_start(out=iev[:], in_=ins["idx_ev"][ch])
            iod = sb.tile([128, g["ODN"] // 16], I16, tag="iod")
            nc.sync.dma_start(out=iod[:], in_=ins["idx_od"][ch])
            eqt = sb.tile([128, 32 * TCH], U16, tag="eq")
            nc.sync.dma_start(out=eqt[:], in_=ins["eq"][ch])
            hg_lo = sb.tile([128, wpc * T_EV, 128], BF16, tag="hglo")
            hg_hi = sb.tile([128, wpc * T_OD, 128], BF16, tag="hghi")
            if os.environ.get("MONET_NO_GATHER", "0") == "1":
                nc.vector.memset(hg_lo[:], 0.5)
                nc.vector.memset(hg_hi[:], 0.5)
            else:
                nc.gpsimd.dma_gather(
                    out_ap=hg_lo[:], in_ap=tbl_ev, idxs_ap=iev[:],
                    num_idxs=g["EVN"], num_idxs_reg=g["EVN"],
                    elem_size=128, elem_step=256, single_packet=False)
                nc.gpsimd.dma_gather(
                    out_ap=hg_hi[:], in_ap=tbl_od, idxs_ap=iod[:],
                    num_idxs=g["ODN"], num_idxs_reg=g["ODN"],
                    elem_size=128, elem_step=256, single_packet=False)
            s3 = sb.tile([128, KK, 32, TCH], BF16, tag="s3")
            eqv = eqt[:].bitcast(BF16).rearrange("p (j t) -> p j t", t=TCH)
            for k in range(KK):
                nc.vector.tensor_tensor(
                    out=s3[:, k], in0=eqv,
                    in1=gauss[:, k, ch * TCH:(ch + 1) * TCH]
                        .rearrange("p (o t) -> p o t", o=1)
                        .broadcast_to([128, 32, TCH]),
                    op=AluOp.mult)
            for wl in range(wpc):
                win = ps.tile([65, KK * 32], F32, tag="ps")
                for tti in range(TPW):
                    tloc = wl * TPW + tti
                    if tti < T_EV:
                        lhs = hg_lo[:, wl * T_EV + tti, 0:65]
                    else:
                        lhs = hg_hi[:, wl * T_OD + (tti - T_EV), 0:65]
                    nc.tensor.matmul(out=win[:], lhsT=lhs,
                                     rhs=s3[:, :, :, tloc],
                                     start=(tti == 0), stop=(tti == TPW - 1))
                sub = wl % 4
                if sub == 0:
                    ust = sb.tile([65, KK, 4, 32], F32, tag="ust")
                nc.scalar.copy(
                    out=ust[:, :, sub, :],
                    in_=win[:].rearrange("u (k j) -> u k j", j=32))
                if sub == 3:
                    gi = (ch * wpc + wl) // 4
                    ap_ = ps.tile([128, OUT], F32, tag="ps")
                    for k in range(KK):
                        lhsu = ust[:, k].rearrange("u a b -> u (a b)")
                        nc.tensor.matmul(
                            out=ap_[:], lhsT=lhsu,
                            rhs=fcwb[:, k * OUT:(k + 1) * OUT],
                            start=(k == 0), stop=(k == KK - 1))
                    nc.scalar.copy(out=agg[:, gi * OUT:(gi + 1) * OUT], in_=ap_[:])

        # ---- BN stats (sum / sumsq over slots via ones-matmul) + AllReduce
        sq = sb1.tile([128, NG * OUT], F32, tag="sq")
        nc.scalar.square(sq[:], agg[:])
        sump = ps.tile([OUT, 1], F32, tag="ps")
        sqp = ps.tile([OUT, 1], F32, tag="ps")
        for gi in range(NG):
            nc.tensor.matmul(out=sump[:], lhsT=agg[:, gi * OUT:(gi + 1) * OUT],
                             rhs=onescol[:], start=(gi == 0), stop=(gi == NG - 1))
            nc.tensor.matmul(out=sqp[:], lhsT=sq[:, gi * OUT:(gi + 1) * OUT],
                             rhs=onescol[:], start=(gi == 0), stop=(gi == NG - 1))
        stats = sb1.tile([OUT, 2], F32, tag="stats")
        nc.scalar.copy(out=stats[:, 0:1], in_=sump[:])
        nc.scalar.copy(out=stats[:, 1:2], in_=sqp[:])
        nc.sync.dma_start(out=stats_in[0:OUT, :], in_=stats[:])
        if NO_CC:
            nc.sync.dma_start(out=stats_out[0:OUT, :], in_=stats_in[0:OUT, :])
        else:
            nc.gpsimd.collective_compute(
                "AllReduce", AluOp.add, replica_groups=[list(range(NCORES))],
                ins=[stats_in[:].opt()], outs=[stats_out[:].opt()])
        stats_ar = sb1.tile([OUT, 2], F32, tag="statsar")
        nc.sync.dma_start(out=stats_ar[:], in_=stats_out[0:OUT, :])
        trp0 = ps.tile([1, OUT], F32, tag="ps")
        nc.tensor.matmul(out=trp0[:], lhsT=stats_ar[:, 0:1],
                         rhs=ident[0:OUT, 0:OUT], start=True, stop=True)
        trp1 = ps.tile([1, OUT], F32, tag="ps")
        nc.tensor.matmul(out=trp1[:], lhsT=stats_ar[:, 1:2],
                         rhs=ident[0:OUT, 0:OUT], start=True, stop=True)
        mean = sb1.tile([1, OUT], F32, tag="mean")
        nc.vector.tensor_scalar(mean[:], trp0[:], 1.0 / nn, None, AluOp.mult)
        ev2 = sb1.tile([1, OUT], F32, tag="ev2")
        nc.vector.tensor_scalar(ev2[:], trp1[:], 1.0 / nn, None, AluOp.mult)
        m2 = sb1.tile([1, OUT], F32, tag="m2")
        nc.vector.tensor_tensor(out=m2[:], in0=mean[:], in1=mean[:], op=AluOp.mult)
        var = sb1.tile([1, OUT], F32, tag="var")
        nc.vector.tensor_tensor(out=var[:], in0=ev2[:], in1=m2[:], op=AluOp.subtract)
        nc.vector.tensor_scalar(var[:], var[:], EPS, None, AluOp.add)
        std = sb1.tile([1, OUT], F32, tag="std")
        nc.scalar.sqrt(std[:], var[:])
        rstd = sb1.tile([1, OUT], F32, tag="rstd")
        nc.vector.reciprocal(rstd[:], std[:])
        bng = sb1.tile([1, OUT], F32, tag="bng")
        bnb = sb1.tile([1, OUT], F32, tag="bnb")
        if last:
            nc.sync.dma_start(out=bng[:], in_=ins["bn_g_l"][:])
            nc.sync.dma_start(out=bnb[:], in_=ins["bn_b_l"][:])
        else:
            nc.sync.dma_start(out=bng[:], in_=ins["bn_g"][li])
            nc.sync.dma_start(out=bnb[:], in_=ins["bn_b"][li])
        sg = sb1.tile([1, OUT], F32, tag="sg")
        nc.vector.tensor_tensor(out=sg[:], in0=rstd[:], in1=bng[:], op=AluOp.mult)
        c0 = sb1.tile([1, OUT], F32, tag="c0")
        nc.vector.tensor_tensor(out=c0[:], in0=mean[:], in1=sg[:], op=AluOp.mult)
        crow = sb1.tile([1, OUT], F32, tag="crow")
        nc.vector.tensor_tensor(out=crow[:], in0=bnb[:], in1=c0[:], op=AluOp.subtract)
        reps = []
        for rsrc in (sg, crow):
            rp = ps.tile([128, OUT], F32, tag="ps")
            nc.tensor.matmul(out=rp[:], lhsT=onesrow[:], rhs=rsrc[:],
                             start=True, stop=True)
            rt = sb1.tile([128, OUT], F32, tag=f"rep{len(reps)}")
            nc.scalar.copy(out=rt[:], in_=rp[:])
            reps.append(rt)

        def rep_b(rt):
            return rt[:].rearrange("p (o c) -> p o c", o=1).broadcast_to([128, NG, OUT])

        bn = sq  # reuse buffer
        aggv = agg[:].rearrange("p (g c) -> p g c", c=OUT)
        bnv = bn[:].rearrange("p (g c) -> p g c", c=OUT)
        nc.vector.tensor_tensor(out=bnv, in0=aggv, in1=rep_b(reps[0]), op=AluOp.mult)
        nc.vector.tensor_tensor(out=bnv, in0=bnv, in1=rep_b(reps[1]), op=AluOp.add)
        nc.vector.tensor_scalar(bn[:], bn[:], 0.0, None, AluOp.max)

        if last:
            nc.sync.dma_start(out=outs["out"][:], in_=bn[:])
        else:
            h_new = sb.tile([128, NG * HID], F32, tag="h")
            nc.vector.tensor_tensor(out=h_new[:], in0=bn[:], in1=h_cur[:],
                                    op=AluOp.add)
            h_cur = h_new
            push_table(h_cur[:])

    stack.close()


# ---------------------------------------------------------------------------
# top-level entry
# ---------------------------------------------------------------------------

def _make_in_maps(g, weights):
    in_maps = []
    for c in range(NCORES):
        pc = g["per_core"][c]
        m = dict(weights)
        m["featT"] = g["featT"][c]
        m["ident"] = np.eye(g["hid"], dtype=np.float32)
        m["idx_ev"] = pc["idx_ev"]
        m["idx_od"] = pc["idx_od"]
        m["eq"] = pc["eq"]
        m["dr"] = pc["dr"]
        m["dc"] = pc["dc"]
        in_maps.append({k + "_d": v for k, v in m.items()})
    return in_maps


def _weights_dict(inputs, g):
    f32 = lambda x: np.ascontiguousarray(np.asarray(x, np.float32))
    w = dict(
        emb_w=f32(inputs["emb_w"]),                       # [128, 64]
        emb_b=f32(inputs["emb_b"]).reshape(1, -1),
        fc_w=f32(inputs["fc_w"]),                         # [3, 64, 192]
        fc_b=f32(inputs["fc_b"]).reshape(g["nhl"], 1, -1),
        mu=f32(inputs["mu"]).reshape(g["nhl"], 1, -1),
        inv_sigma=f32(inputs["inv_sigma"]).reshape(g["nhl"], 1, -1),
        pp_w=f32(inputs["pp_w"]).reshape(g["nhl"], 1, -1),
        pp_b=f32(inputs["pp_b"]).reshape(g["nhl"], 1, -1),
        bn_g=f32(inputs["bn_g"]).reshape(g["nhl"], 1, -1),
        bn_b=f32(inputs["bn_b"]).reshape(g["nhl"], 1, -1),
        fc_w_l=f32(inputs["fc_w_l"]), fc_b_l=f32(inputs["fc_b_l"]).reshape(1, -1),
        mu_l=f32(inputs["mu_l"]).reshape(1, -1),
        inv_sigma_l=f32(inputs["inv_sigma_l"]).reshape(1, -1),
        pp_w_l=f32(inputs["pp_w_l"]).reshape(1, -1),
        pp_b_l=f32(inputs["pp_b_l"]).reshape(1, -1),
        bn_g_l=f32(inputs["bn_g_l"]).reshape(1, -1),
        bn_b_l=f32(inputs["bn_b_l"]).reshape(1, -1),
    )
    return w


def _build_featT(inputs, g):
    feat = np.asarray(inputs["feature"], np.float32)
    featT = []
    for c in range(NCORES):
        arr = np.zeros((g["in_dim"], g["npc"]), np.float32)
        nds = np.flatnonzero(g["core_of"] == c)
        arr[:, g["slot_of"][nds] % g["npc"]] = feat[nds].T
        featT.append(arr)
    g["featT"] = featT


def run_device(g, weights, trace=False):
    nc = bacc.Bacc("TRN2", target_bir_lowering=False, debug=False,
                   num_devices=NCORES)
    ins_ap, outs_ap = {}, {}
    in_maps = _make_in_maps(g, weights)
    for name, arr in in_maps[0].items():
        t = nc.dram_tensor(name, list(arr.shape), mybir.dt.from_np(arr.dtype),
                           kind="ExternalInput")
        ins_ap[name[:-2]] = t.ap()
    out_t = nc.dram_tensor("out_d", [128, g["NG"] * g["ncls"]], F32,
                           kind="ExternalOutput")
    outs_ap["out"] = out_t.ap()

    with tile.TileContext(nc) as tc:
        build(tc, outs_ap, ins_ap, g)
    nc.compile()

    res = bass_utils.run_bass_kernel_spmd(
        nc, in_maps, core_ids=list(range(NCORES)), trace=trace)
    return res


def assemble_output(g, res):
    out = np.zeros((g["n"], g["ncls"]), np.float32)
    for c in range(NCORES):
        oc = res.results[c]["out_d"].reshape(128, g["NG"], g["ncls"])
        nds = np.flatnonzero(g["core_of"] == c)
        sl = g["slot_of"][nds] % g["npc"]
        out[nds] = oc[sl % 128, sl // 128, :]
    return out


def kernel(**inputs):
    g = preprocess(np.asarray(inputs["edge_index"]), GEOM_REAL)
    _build_featT(inputs, g)
    weights = _weights_dict(inputs, g)
    res = run_device(g, weights, trace=os.environ.get("MONET_TRACE", "0") == "1")
    out = assemble_output(g, res)
    kernel.last_exec_time_ns = getattr(res, "exec_time_ns", None)
    return out


# ---------------------------------------------------------------------------
# numpy reference (dev only; mirrors reference.py)
# ---------------------------------------------------------------------------

def numpy_reference(inputs, n, nhl=3):
    f = {k: np.asarray(v, np.float64 if np.asarray(v).dtype.kind == "f" else None)
         for k, v in inputs.items()}
    row, col = np.asarray(inputs["edge_index"][0]), np.asarray(inputs["edge_index"][1])
    deg_r = np.bincount(row, minlength=n)
    deg_c = np.bincount(col, minlength=n)
    srcs = 1.0 / np.sqrt(deg_r[row] + 1.0)
    dsts = 1.0 / np.sqrt(deg_c[col] + 1.0)
    pseudo = np.stack([srcs, dsts], -1)
    h = f["feature"] @ f["emb_w"] + f["emb_b"]

    def gmm(h, psd, fcw, fcb, mu, isg, bng, bnb, residual):
        kk, out = mu.shape[0], fcw.shape[1] // mu.shape[0]
        hp = (h @ fcw + fcb).reshape(n, kk, out)
        diff = psd[:, None, :] - mu
        gauss = np.exp(-0.5 * np.sum((diff * isg) ** 2, -1))
        msg = np.einsum("ek,ekc->ec", gauss, hp[row])
        agg = np.zeros((n, out))
        np.add.at(agg, col, msg)
        mean = agg.mean(0)
        var = agg.var(0)
        hbn = (agg - mean) / np.sqrt(var + EPS) * bng + bnb
        hnew = np.maximum(hbn, 0.0)
        return h + hnew if residual else hnew

    for i in range(nhl):
        psd = np.tanh(pseudo @ f["pp_w"][i] + f["pp_b"][i])
        h = gmm(h, psd, f["fc_w"][i], f["fc_b"][i], f["mu"][i],
                f["inv_sigma"][i], f["bn_g"][i], f["bn_b"][i], True)
    psd = np.tanh(pseudo @ f["pp_w_l"] + f["pp_b_l"])
    h = gmm(h, psd, f["fc_w_l"], f["fc_b_l"], f["mu_l"], f["inv_sigma_l"],
            f["bn_g_l"], f["bn_b_l"], False)
    return h.astype(np.float32)


# ---------------------------------------------------------------------------
# timed execution (repeated PJRT calls on a single compiled executable)
# ---------------------------------------------------------------------------

def run_device_timed(g, weights, n_iters=5):
    import time
    import jax
    from jax.sharding import Mesh, PartitionSpec
    from jax.experimental.shard_map import shard_map
    from concourse import bass2jax as b2j

    nc = bacc.Bacc("TRN2", target_bir_lowering=False, debug=False,
                   num_devices=NCORES)
    ins_ap = {}
    in_maps = _make_in_maps(g, weights)
    for name, arr in in_maps[0].items():
        t = nc.dram_tensor(name, list(arr.shape), mybir.dt.from_np(arr.dtype),
                           kind="ExternalInput")
        ins_ap[name[:-2]] = t.ap()
    out_t = nc.dram_tensor("out_d", [128, g["NG"] * g["ncls"]], F32,
                           kind="ExternalOutput")
    outs_ap = {"out": out_t.ap()}
    with tile.TileContext(nc) as tc:
        build(tc, outs_ap, ins_ap, g)
    nc.compile()

    b2j.install_neuronx_cc_hook()
    partition_name = (nc.partition_id_tensor.name
                      if nc.partition_id_tensor else None)
    in_names, out_names, out_avals, zero_outs = [], [], [], []
    for alloc in nc.m.functions[0].allocations:
        if not isinstance(alloc, mybir.MemoryLocationSet):
            continue
        name = alloc.memorylocations[0].name
        if alloc.kind == "ExternalInput":
            if name != partition_name:
                in_names.append(name)
        elif alloc.kind == "ExternalOutput":
            dt = mybir.dt.np(alloc.dtype)
            out_avals.append(jax.core.ShapedArray(tuple(alloc.tensor_shape), dt))
            out_names.append(name)
            zero_outs.append(np.zeros(tuple(alloc.tensor_shape), dt))
    n_params = len(in_names)
    n_outs = len(out_names)
    in_names = in_names + out_names
    if partition_name is not None:
        in_names.append(partition_name)
    donate = tuple(range(n_params, n_params + n_outs))

    def _body(*args):
        operands = list(args)
        if partition_name is not None:
            operands.append(b2j.partition_id_tensor())
        outs = b2j._bass_exec_p.bind(
            *operands,
            out_avals=tuple(out_avals),
            in_names=tuple(in_names),
            out_names=tuple(out_names),
            lowering_input_output_aliases=(),
            sim_require_finite=True,
            sim_require_nnan=True,
            nc=nc,
        )
        return tuple(outs)

    devices = jax.devices()[:NCORES]
    mesh = Mesh(np.asarray(devices), ("core",))
    sharded = jax.jit(
        shard_map(_body, mesh=mesh,
                  in_specs=(PartitionSpec("core"),) * (n_params + n_outs),
                  out_specs=(PartitionSpec("core"),) * n_outs,
                  check_rep=False),
        donate_argnums=donate, keep_unused=True)
    per_core = [[np.asarray(m[nm]) for nm in in_names[:n_params]]
                for m in in_maps]
    concat_in = [np.concatenate([per_core[c][i] for c in range(NCORES)], 0)
                 for i in range(n_params)]
    concat_in = [jax.device_put(a) for a in concat_in]

    times = []
    out_arrs = None
    for it in range(n_iters):
        czeros = [np.zeros((NCORES * z.shape[0], *z.shape[1:]), z.dtype)
                  for z in zero_outs]
        t0 = time.perf_counter()
        out_arrs = sharded(*concat_in, *czeros)
        jax.block_until_ready(out_arrs)
        times.append(time.perf_counter() - t0)
    results = [
        {nm: np.asarray(out_arrs[i]).reshape(NCORES, *out_avals[i].shape)[c]
         for i, nm in enumerate(out_names)}
        for c in range(NCORES)
    ]

    class R:
        pass
    r = R()
    r.results = results
    r.exec_time_ns = int(min(times[1:]) * 1e9) if len(times) > 1 else None
    r.all_times = times
    return r



# revision 6
# speedup vs baseline: 39.0913x; 39.0913x over previous
"""MoNet (GMM graph conv) on Trainium2 — 8-core SPMD Bass/Tile kernel.

Sharding: dst-node slices per core (edge-parallel within core), with node
relabeling into per-core "slot space". Uniform SPMD program; per-core data.

v2 layout: gapless edge tiles. Per (chunk, window, class) a GLOBAL capacity
(max edge count over cores) reserves a contiguous position range; positions
pack into 128-wide gather tiles that may straddle window boundaries. A
(tile, window) pair is a "segment": the PE matmul for window w over a
boundary tile uses an s3 block whose other-window partitions are zero.
 - gather: dma_gather 256B rows (64 bf16 feats + ones col + pad) via two
   stride-2 table views (int16 idx range), indices sorted within each
   (window, class) run for HBM locality.
 - s3 = one-hot(dst slot) x gauss built by DVE from uint8 one-hot.
 - PSUM: 4 windows share one bank [65, 4, K*32]; one Act copy stages all 4
   to bf16 ust; fc matmuls in bf16.
 - BN stats via DVE group-reduce + 1-col PE matmuls + AllReduce; h pushed
   compact (64 cols bf16) through a Shared-output AllGather into the
   256B-row gather table (ones column initialized once).
"""
import sys, os
import numpy as np

if "/opt/trn_rl_repo" not in sys.path:
    sys.path.insert(0, "/opt/trn_rl_repo")

import ml_dtypes
from concourse import bass, bacc, mybir, tile
from concourse import bass_utils

AluOp = mybir.AluOpType
Act = mybir.ActivationFunctionType
F32 = mybir.dt.float32
BF16 = mybir.dt.bfloat16
I16 = mybir.dt.int16
U8 = mybir.dt.uint8

NCORES = 8
EPS = 1e-5

GEOM_REAL = dict(n=50000, e=800000, in_dim=128, hid=64, k=3, pdim=2,
                 ncls=16, nhl=3, W=208, wpchunk=16)


def derive(geom):
    g = dict(geom)
    g["npc"] = g["W"] * 32                # dst slots per core
    g["NG"] = g["npc"] // 128             # 128-slot groups per core
    g["NCH"] = g["W"] // g["wpchunk"]     # chunks per layer
    g["n_rows"] = NCORES * g["npc"]       # table rows
    assert g["n_rows"] // 2 <= 32767
    return g


# ---------------------------------------------------------------------------
# host preprocessing (pure integer/index manipulation)
# ---------------------------------------------------------------------------

def preprocess(edge_index, geom):
    g = derive(geom)
    n, W, npc = g["n"], g["W"], g["npc"]
    NCH, wpc = g["NCH"], g["wpchunk"]
    row = np.asarray(edge_index[0], np.int64)
    col = np.asarray(edge_index[1], np.int64)
    deg_r = np.bincount(row, minlength=n).astype(np.int64)
    deg_c = np.bincount(col, minlength=n).astype(np.int64)

    # 1) nodes -> cores (snake deal by in-degree for balanced edge counts)
    order = np.argsort(-deg_c, kind="stable")
    core_of = np.empty(n, np.int64)
    blk = np.arange(n) // NCORES
    pos = np.arange(n) % NCORES
    snake = np.where(blk % 2 == 0, pos, NCORES - 1 - pos)
    core_of[order] = snake

    # 2) class A (even table rows) = per-core top half by out-degree
    is_a = np.zeros(n, bool)
    for c in range(NCORES):
        nds = np.flatnonzero(core_of == c)
        half = min((len(nds) + 1) // 2, W * 16)
        topa = nds[np.argsort(-deg_r[nds], kind="stable")][:half]
        is_a[topa] = True

    src_a = is_a[row]
    in_ev = np.bincount(col[src_a], minlength=n).astype(np.int64)
    in_od = np.bincount(col[~src_a], minlength=n).astype(np.int64)

    # 3) per-core window packing (first-fit decreasing, loose caps only to
    #    balance chunks) then window relabel by total load so that per-rank
    #    counts align across cores (tight global caps).
    cap_ev, cap_od = 6 * 128, 4 * 128
    slot_of = np.full(n, -1, np.int64)
    wcnt_ev = np.zeros((NCORES, W), np.int64)
    wcnt_od = np.zeros((NCORES, W), np.int64)
    for c in range(NCORES):
        nds = np.flatnonzero(core_of == c)
        nds = nds[np.argsort(-(in_ev[nds] + in_od[nds]), kind="stable")]
        wev = np.zeros(W, np.int64); wod = np.zeros(W, np.int64)
        wna = np.zeros(W, np.int64); wnb = np.zeros(W, np.int64)
        wslot = np.full(n, -1, np.int64)
        for nd in nds:
            a = bool(is_a[nd])
            for w in range(W):
                if a and wna[w] >= 16: continue
                if (not a) and wnb[w] >= 16: continue
                if wev[w] + in_ev[nd] > cap_ev: continue
                if wod[w] + in_od[nd] > cap_od: continue
                if a:
                    j = 2 * wna[w]; wna[w] += 1
                else:
                    j = 2 * wnb[w] + 1; wnb[w] += 1
                wev[w] += in_ev[nd]; wod[w] += in_od[nd]
                wslot[nd] = w * 32 + j
                break
            else:
                raise RuntimeError(f"window packing failed (core {c})")
        # relabel windows: sort by (ev+od) load descending so rank k has the
        # k-th largest load on every core -> per-rank max over cores ~= mean
        perm = np.argsort(-(wev + wod), kind="stable")   # rank -> old w
        rank_of = np.empty(W, np.int64)
        # deal ranks round-robin over chunks: rank r -> window
        # (r % NCH)*wpc + r//NCH, so every chunk gets a balanced mix and
        # per-rank loads still align across cores
        rr = (np.arange(W) % NCH) * wpc + np.arange(W) // NCH
        rank_of[perm] = rr
        sel = wslot >= 0
        slot_of[sel] = (core_of[sel] * npc + rank_of[wslot[sel] // 32] * 32
                        + wslot[sel] % 32)
        wcnt_ev[c][rr] = wev[perm]
        wcnt_od[c][rr] = wod[perm]

    assert (slot_of >= 0).all()
    assert (slot_of[is_a] % 2 == 0).all() and (slot_of[~is_a] % 2 == 1).all()

    g.update(core_of=core_of, slot_of=slot_of, deg_r=deg_r, deg_c=deg_c)

    # 4) global segment template from per-(window, class) caps
    caps_ev = wcnt_ev.max(axis=0)        # [W]
    caps_od = wcnt_od.max(axis=0)
    EVN = np.zeros(NCH, np.int64)        # gather idx count per chunk (even)
    ODN = np.zeros(NCH, np.int64)
    NSEG = np.zeros(NCH, np.int64)
    seg_template = []                    # per ch: dict keyed (wl, cls, tile)->col
    base_ev = np.zeros(W, np.int64)      # position base of window's run
    base_od = np.zeros(W, np.int64)
    win_segs = []                        # per ch: list over wl of [(cls, tile, col)]
    for ch in range(NCH):
        ws = range(ch * wpc, (ch + 1) * wpc)
        segcol = {}
        wl_segs = [[] for _ in range(wpc)]
        for cls, caps, base_arr, tot in ((0, caps_ev, base_ev, None),
                                         (1, caps_od, base_od, None)):
            p0 = 0
            for wl, w in enumerate(ws):
                base_arr[w] = p0
                cap = int(caps[w])
                if cap > 0:
                    for t in range(p0 // 128, (p0 + cap - 1) // 128 + 1):
                        key = (wl, cls, t)
                        if key not in segcol:
                            segcol[key] = -1   # placeholder, ordered later
                        wl_segs[wl].append((cls, t))
                p0 += cap
            if cls == 0:
                EVN[ch] = -(-p0 // 128) * 128
            else:
                ODN[ch] = -(-p0 // 128) * 128
        # assign columns ordered by (wl, cls, tile)
        cols = 0
        for wl in range(wpc):
            newl = []
            for cls, t in wl_segs[wl]:
                segcol[(wl, cls, t)] = cols
                newl.append((cls, t, cols))
                cols += 1
            wl_segs[wl] = newl
        NSEG[ch] = cols
        seg_template.append(segcol)
        win_segs.append(wl_segs)

    SEG_CAP = int(NSEG.max())
    EVT_CAP = int((EVN // 128).max())
    ODT_CAP = int((ODN // 128).max())
    g.update(EVN=EVN.tolist(), ODN=ODN.tolist(), NSEG=NSEG.tolist(),
             SEG_CAP=SEG_CAP, EVT_CAP=EVT_CAP, ODT_CAP=ODT_CAP,
             win_segs=win_segs, SEGT=NCH * SEG_CAP)
    tot_slots = int(EVN.sum() + ODN.sum())
    g["gather_slots"] = tot_slots

    # 5) per-core arrays
    e_core = core_of[col]
    e_slot = slot_of[col] % npc
    e_w = e_slot // 32
    e_j = e_slot % 32
    e_view = (slot_of[row] // 2).astype(np.int64)   # stride-2 view index

    per_core = []
    for c in range(NCORES):
        idx_ev = np.zeros((NCH, 16, EVT_CAP * 8), np.int16)
        idx_od = np.zeros((NCH, 16, ODT_CAP * 8), np.int16)
        eq = np.zeros((NCH, 128, SEG_CAP * 32), np.uint8)
        dr = np.zeros((128, NCH * SEG_CAP), np.float32)
        dc = np.zeros((128, NCH * SEG_CAP), np.float32)

        sel = np.flatnonzero(e_core == c)
        ew, ej, ecls = e_w[sel], e_j[sel], src_a[sel]
        evi = e_view[sel]
        edr = deg_r[row[sel]].astype(np.float32)
        edc = deg_c[col[sel]].astype(np.float32)
        # group edges by (window, class); sort by view index inside a group
        okey = ew * 2 + (~ecls).astype(np.int64)
        eorder = np.lexsort((evi, okey))
        bnd = np.searchsorted(okey[eorder], np.arange(2 * W + 1))
        for w in range(W):
            ch, wl = divmod(w, wpc)
            segcol = seg_template[ch]
            for cls in (0, 1):
                kk = w * 2 + cls
                eids = eorder[bnd[kk]:bnd[kk + 1]]
                ne = len(eids)
                if ne == 0:
                    continue
                b = base_ev[w] if cls == 0 else base_od[w]
                q = np.arange(ne)
                pp = (b + q) % 128
                tt = (b + q) // 128
                scol = np.array([segcol[(wl, cls, int(t))] for t in tt])
                tgt = idx_ev if cls == 0 else idx_od
                tgt[ch][(b + q) % 16, (b + q) // 16] = evi[eids].astype(np.int16)
                eq[ch][pp, scol * 32 + ej[eids]] = 1
                dr[pp, ch * SEG_CAP + scol] = edr[eids]
                dc[pp, ch * SEG_CAP + scol] = edc[eids]
        idx_ev = np.tile(idx_ev, (1, 8, 1))
        idx_od = np.tile(idx_od, (1, 8, 1))
        per_core.append(dict(idx_ev=idx_ev, idx_od=idx_od, eq=eq, dr=dr, dc=dc))
    g["per_core"] = per_core
    return g


# ---------------------------------------------------------------------------
# device program
# ---------------------------------------------------------------------------

def build(tc, outs, ins, g):
    nc = tc.nc
    W, npc, NG = g["W"], g["npc"], g["NG"]
    NCH, wpc = g["NCH"], g["wpchunk"]
    HID, KK, NCLS, NHL = g["hid"], g["k"], g["ncls"], g["nhl"]
    n_rows = g["n_rows"]
    nn = g["n"]
    SEG_CAP, SEGT = g["SEG_CAP"], g["SEGT"]
    EVT_CAP, ODT_CAP = g["EVT_CAP"], g["ODT_CAP"]
    EVN, ODN, NSEG = g["EVN"], g["ODN"], g["NSEG"]
    win_segs = g["win_segs"]

    import contextlib
    stack = contextlib.ExitStack()
    sbc = stack.enter_context(tc.tile_pool(name="sbc", bufs=1))
    sb1 = stack.enter_context(tc.tile_pool(name="sb1", bufs=1))
    sb = stack.enter_context(tc.tile_pool(name="sb", bufs=2))
    ps = stack.enter_context(tc.tile_pool(name="ps", bufs=8, space="PSUM"))
    dram = stack.enter_context(tc.tile_pool(name="dram", bufs=1, space="DRAM"))

    # ---- constants / persistent state
    onesrow = sbc.tile([1, 128], F32); nc.vector.memset(onesrow[:], 1.0)
    onescol = sbc.tile([128, 1], F32); nc.vector.memset(onescol[:], 1.0)
    ident = sbc.tile([HID, HID], F32)
    nc.sync.dma_start(out=ident[:], in_=ins["ident"][:])
    stage = sbc.tile([128, NG, 128], BF16)
    nc.vector.memset(stage[:], 0.0)
    nc.vector.memset(stage[:, :, 64:65], 1.0)
    srcs = sbc.tile([128, SEGT], F32)
    dsts = sbc.tile([128, SEGT], F32)
    gauss = sbc.tile([128, KK, SEGT], BF16)

    table = dram.tile([n_rows, 128], BF16)
    stage_d = dram.tile([npc, 128], BF16)
    stats_in = dram.tile([HID, 2], F32)
    stats_out = dram.tile([HID, 2], F32)

    zz = sbc.tile([HID, 2], F32)
    nc.vector.memset(zz[:], 0.0)
    nc.sync.dma_start(out=stats_in[:], in_=zz[:])
    nc.sync.dma_start(out=stats_out[:], in_=zz[:])

    tbl_ev = table[:].rearrange("(m two) c -> m (two c)", two=2)[:, 0:128]
    tbl_od = table[:].rearrange("(m two) c -> m (two c)", two=2)[:, 128:256]

    # ---- prologue: pseudo coords
    with tc.tile_pool(name="pro", bufs=1) as pro:
        drt = pro.tile([128, SEGT], F32)
        nc.sync.dma_start(out=drt[:], in_=ins["dr"][:])
        dct = pro.tile([128, SEGT], F32)
        nc.sync.dma_start(out=dct[:], in_=ins["dc"][:])
        t0 = pro.tile([128, SEGT], F32)
        for dsrc, dout in ((drt, srcs), (dct, dsts)):
            nc.vector.tensor_scalar(t0[:], dsrc[:], 1.0, None, AluOp.add)
            nc.scalar.sqrt(t0[:], t0[:])
            nc.vector.reciprocal(dout[:], t0[:])

    NO_CC = os.environ.get("MONET_NO_CC", "0") == "1"
    NHID_RUN = int(os.environ.get("MONET_NLAYERS", str(NHL)))
    NREPEAT = int(os.environ.get("MONET_REPEAT", "1"))

    def push_table(h_flat):
        # h_flat [128, NG*64] f32 -> bf16 stage (+ones col) -> AllGather
        nc.vector.tensor_copy(
            out=stage[:, :, 0:64],
            in_=h_flat.rearrange("p (g c) -> p g c", c=64))
        nc.sync.dma_start(
            out=stage_d[:].rearrange("(gp p) c -> p gp c", p=128),
            in_=stage[:])
        if NO_CC:
            nc.sync.dma_start(out=table[0:npc, :], in_=stage_d[:])
            return
        nc.gpsimd.collective_compute(
            "AllGather", AluOp.bypass, replica_groups=[list(range(NCORES))],
            ins=[stage_d[:].opt()], outs=[table[:].opt()])

    def one_forward():
        # ---- embed: h0 = featT.T @ emb_w + emb_b
        h_cur = sb.tile([128, NG * HID], F32, tag="h")
        with tc.tile_pool(name="emb", bufs=2) as emb:
            embw = emb.tile([128, HID], F32, tag="embw")
            nc.sync.dma_start(out=embw[:], in_=ins["emb_w"][:])
            ebrow = emb.tile([1, HID], F32, tag="ebrow")
            nc.sync.dma_start(out=ebrow[:], in_=ins["emb_b"][:])
            for gi in range(NG):
                ft = emb.tile([128, 128], F32, tag="ft")
                nc.sync.dma_start(out=ft[:],
                                  in_=ins["featT"][:, gi * 128:(gi + 1) * 128])
                ep = ps.tile([128, HID], F32, tag="ps")
                nc.tensor.matmul(out=ep[:], lhsT=ft[:],
                                 rhs=embw[:], start=True, stop=True)
                nc.scalar.copy(out=h_cur[:, gi * HID:(gi + 1) * HID], in_=ep[:])
            ebp = ps.tile([128, HID], F32, tag="ps")
            nc.tensor.matmul(out=ebp[:], lhsT=onesrow[:], rhs=ebrow[:],
                             start=True, stop=True)
            ebrep = emb.tile([128, HID], F32)
            nc.scalar.copy(out=ebrep[:], in_=ebp[:])
            nc.vector.tensor_tensor(
                out=h_cur[:], in0=h_cur[:],
                in1=ebrep[:].rearrange("p (o c) -> p o c", o=1)
                    .broadcast_to([128, NG, HID]),
                op=AluOp.add)
        push_table(h_cur[:])

        # ---- layers
        for li in list(range(NHID_RUN)) + [NHL]:
            last = li == NHL
            OUT = NCLS if last else HID

            # scalars row: [w00 w01 w10 w11 b0 b1 | mu k*2+d | isg k*2+d]
            scal_row = sb1.tile([1, 32], F32, tag="scalrow")
            nc.vector.memset(scal_row[:], 0.0)
            if last:
                nc.sync.dma_start(out=scal_row[:, 0:4], in_=ins["pp_w_l"][:])
                nc.sync.dma_start(out=scal_row[:, 4:6], in_=ins["pp_b_l"][:])
                nc.sync.dma_start(out=scal_row[:, 6:6 + 2 * KK], in_=ins["mu_l"][:])
                nc.sync.dma_start(out=scal_row[:, 18:18 + 2 * KK],
                                  in_=ins["inv_sigma_l"][:])
            else:
                nc.sync.dma_start(out=scal_row[:, 0:4], in_=ins["pp_w"][li])
                nc.sync.dma_start(out=scal_row[:, 4:6], in_=ins["pp_b"][li])
                nc.sync.dma_start(out=scal_row[:, 6:6 + 2 * KK], in_=ins["mu"][li])
                nc.sync.dma_start(out=scal_row[:, 18:18 + 2 * KK],
                                  in_=ins["inv_sigma"][li])
            scp = ps.tile([128, 32], F32, tag="ps")
            nc.tensor.matmul(out=scp[:], lhsT=onesrow[:], rhs=scal_row[:],
                             start=True, stop=True)
            scal = sb1.tile([128, 32], F32, tag="scal")
            nc.scalar.copy(out=scal[:], in_=scp[:])

            def sc(j):
                return scal[:, j:j + 1]

            # gauss[k] = exp(-0.5*(((ps0-mu_k0)*is_k0)^2+((ps1-mu_k1)*is_k1)^2))
            ps0 = sb1.tile([128, SEGT], F32, tag="ps0")
            ps1 = sb1.tile([128, SEGT], F32, tag="ps1")
            ta = sb1.tile([128, SEGT], F32, tag="ta")
            tb = sb1.tile([128, SEGT], F32, tag="tb")
            for (pst, wA, wB, bB) in ((ps0, 0, 2, 4), (ps1, 1, 3, 5)):
                nc.vector.tensor_scalar(ta[:], srcs[:], sc(wA), None, AluOp.mult)
                nc.vector.tensor_scalar(tb[:], dsts[:], sc(wB), None, AluOp.mult)
                nc.vector.tensor_tensor(out=ta[:], in0=ta[:], in1=tb[:],
                                        op=AluOp.add)
                nc.scalar.activation(pst[:], ta[:], Act.Tanh, bias=sc(bB),
                                     scale=1.0)
            for k in range(KK):
                nc.vector.tensor_scalar(ta[:], ps0[:], sc(6 + 2 * k),
                                        sc(18 + 2 * k),
                                        AluOp.subtract, AluOp.mult)
                nc.vector.tensor_scalar(tb[:], ps1[:], sc(7 + 2 * k),
                                        sc(19 + 2 * k),
                                        AluOp.subtract, AluOp.mult)
                nc.scalar.square(ta[:], ta[:])
                nc.scalar.square(tb[:], tb[:])
                nc.vector.tensor_tensor(out=ta[:], in0=ta[:], in1=tb[:],
                                        op=AluOp.add)
                nc.scalar.activation(gauss[:, k, :], ta[:], Act.Exp,
                                     bias=0.0, scale=-0.5)

            # dense weights [65, K*OUT] bf16 (stacked [w; b] on host)
            fcwb = sb1.tile([65, KK * OUT], F32, tag="fcwb")
            if last:
                nc.sync.dma_start(out=fcwb[:], in_=ins["fc_wb_l"][:])
            else:
                nc.sync.dma_start(out=fcwb[:], in_=ins["fc_wb"][li])

            agg = sb1.tile([128, NG * OUT], F32, tag="aggsb")

            # ---- edge pipeline
            for ch in range(NCH):
                nev_t, nod_t = EVN[ch] // 128, ODN[ch] // 128
                nseg = NSEG[ch]
                FULLG = os.environ.get("MONET_FULLG", "0") == "1"
                evn = EVT_CAP * 128 if FULLG else EVN[ch]
                odn = ODT_CAP * 128 if FULLG else ODN[ch]
                iev = sb.tile([128, EVT_CAP * 8], I16, tag="iev")
                nc.sync.dma_start(out=iev[:, 0:evn // 16],
                                  in_=ins["idx_ev"][ch][:, 0:evn // 16])
                iod = sb.tile([128, ODT_CAP * 8], I16, tag="iod")
                nc.sync.dma_start(out=iod[:, 0:odn // 16],
                                  in_=ins["idx_od"][ch][:, 0:odn // 16])
                eqt = sb.tile([128, SEG_CAP, 32], U8, tag="eq")
                nc.sync.dma_start(
                    out=eqt[:].rearrange("p s j -> p (s j)")[:, 0:nseg * 32],
                    in_=ins["eq"][ch][:, 0:nseg * 32])
                hg_lo = sb.tile([128, EVT_CAP, 128], BF16, tag="hglo")
                hg_hi = sb.tile([128, ODT_CAP, 128], BF16, tag="hghi")
                if os.environ.get("MONET_NO_GATHER", "0") == "1":
                    nc.vector.memset(hg_lo[:], 0.5)
                    nc.vector.memset(hg_hi[:], 0.5)
                else:
                    nc.gpsimd.dma_gather(
                        out_ap=hg_lo[:, 0:evn // 128, :], in_ap=tbl_ev,
                        idxs_ap=iev[:, 0:evn // 16],
                        num_idxs=evn, num_idxs_reg=evn,
                        elem_size=128, elem_step=256,
                        single_packet=os.environ.get("MONET_SP", "0") == "1",
                        queue_num=(2 * ch) % 4)
                    nc.gpsimd.dma_gather(
                        out_ap=hg_hi[:, 0:odn // 128, :], in_ap=tbl_od,
                        idxs_ap=iod[:, 0:odn // 16],
                        num_idxs=odn, num_idxs_reg=odn,
                        elem_size=128, elem_step=256,
                        single_packet=os.environ.get("MONET_SP", "0") == "1",
                        queue_num=(2 * ch + 1) % 4)
                s3 = sb.tile([128, KK, SEG_CAP, 32], BF16, tag="s3")
                for k in range(KK):
                    nc.vector.tensor_tensor(
                        out=s3[:, k, 0:nseg, :], in0=eqt[:, 0:nseg, :],
                        in1=gauss[:, k, ch * SEG_CAP:ch * SEG_CAP + nseg]
                            .rearrange("p (s o) -> p s o", o=1)
                            .broadcast_to([128, nseg, 32]),
                        op=AluOp.mult)
                OLDPSUM = os.environ.get("MONET_OLDPSUM", "0") == "1"
                for wl in range(wpc):
                    sub = wl % 4
                    if OLDPSUM:
                        win = ps.tile([65, KK * 32], F32, tag="ps")
                    elif sub == 0:
                        winq = ps.tile([65, 4, KK * 32], F32, tag="ps")
                    segs = win_segs[ch][wl]
                    if not segs:
                        nc.vector.memset(
                            (win[:] if OLDPSUM else winq[:, sub, :]), 0.0)
                    for si, (cls, t, scol) in enumerate(segs):
                        lhs = (hg_lo if cls == 0 else hg_hi)[:, t, 0:65]
                        nc.tensor.matmul(
                            out=(win[:] if OLDPSUM else winq[:, sub, :]),
                            lhsT=lhs,
                            rhs=s3[:, :, scol, :],
                            start=(si == 0), stop=(si == len(segs) - 1))
                    if OLDPSUM:
                        if sub == 0:
                            ust = sb.tile([65, KK, 4, 32], F32, tag="ust")
                        nc.scalar.copy(
                            out=ust[:, :, sub, :],
                            in_=win[:].rearrange("u (k j) -> u k j", j=32))
                    if sub == 3:
                        gi = (ch * wpc + wl) // 4
                        if not OLDPSUM:
                            ust = sb.tile([65, KK, 4, 32], F32, tag="ust")
                            nc.vector.tensor_copy(
                                out=ust[:].rearrange("u k s j -> u s k j"),
                                in_=winq[:].rearrange("u s (k j) -> u s k j",
                                                      j=32))
                        ap_ = ps.tile([128, OUT], F32, tag="ps")
                        for k in range(KK):
                            lhsu = ust[:, k].rearrange("u s j -> u (s j)")
                            nc.tensor.matmul(
                                out=ap_[:], lhsT=lhsu,
                                rhs=fcwb[:, k * OUT:(k + 1) * OUT],
                                start=(k == 0), stop=(k == KK - 1))
                        nc.scalar.copy(out=agg[:, gi * OUT:(gi + 1) * OUT],
                                       in_=ap_[:])

            # ---- BN stats: group-reduce on DVE, cross-partition via 1-col PE
            sq = sb1.tile([128, NG * OUT], F32, tag="sq")
            nc.scalar.square(sq[:], agg[:])
            aggr = sb1.tile([128, OUT], F32, tag="aggr")
            nc.vector.tensor_reduce(
                out=aggr[:], in_=agg[:].rearrange("p (g c) -> p c g", c=OUT),
                axis=mybir.AxisListType.X, op=AluOp.add)
            sqr = sb1.tile([128, OUT], F32, tag="sqr")
            nc.vector.tensor_reduce(
                out=sqr[:], in_=sq[:].rearrange("p (g c) -> p c g", c=OUT),
                axis=mybir.AxisListType.X, op=AluOp.add)
            sump = ps.tile([OUT, 1], F32, tag="ps")
            nc.tensor.matmul(out=sump[:], lhsT=aggr[:], rhs=onescol[:],
                             start=True, stop=True)
            sqp = ps.tile([OUT, 1], F32, tag="ps")
            nc.tensor.matmul(out=sqp[:], lhsT=sqr[:], rhs=onescol[:],
                             start=True, stop=True)
            stats = sb1.tile([OUT, 2], F32, tag="stats")
            nc.scalar.copy(out=stats[:, 0:1], in_=sump[:])
            nc.scalar.copy(out=stats[:, 1:2], in_=sqp[:])
            nc.sync.dma_start(out=stats_in[0:OUT, :], in_=stats[:])
            if NO_CC:
                nc.sync.dma_start(out=stats_out[0:OUT, :], in_=stats_in[0:OUT, :])
            else:
                nc.gpsimd.collective_compute(
                    "AllReduce", AluOp.add,
                    replica_groups=[list(range(NCORES))],
                    ins=[stats_in[:].opt()], outs=[stats_out[:].opt()])
            stats_ar = sb1.tile([OUT, 2], F32, tag="statsar")
            nc.sync.dma_start(out=stats_ar[:], in_=stats_out[0:OUT, :])
            trp0 = ps.tile([1, OUT], F32, tag="ps")
            nc.tensor.matmul(out=trp0[:], lhsT=stats_ar[:, 0:1],
                             rhs=ident[0:OUT, 0:OUT], start=True, stop=True)
            trp1 = ps.tile([1, OUT], F32, tag="ps")
            nc.tensor.matmul(out=trp1[:], lhsT=stats_ar[:, 1:2],
                             rhs=ident[0:OUT, 0:OUT], start=True, stop=True)
            mean = sb1.tile([1, OUT], F32, tag="mean")
            nc.vector.tensor_scalar(mean[:], trp0[:], 1.0 / nn, None, AluOp.mult)
            ev2 = sb1.tile([1, OUT], F32, tag="ev2")
            nc.vector.tensor_scalar(ev2[:], trp1[:], 1.0 / nn, None, AluOp.mult)
            m2 = sb1.tile([1, OUT], F32, tag="m2")
            nc.vector.tensor_tensor(out=m2[:], in0=mean[:], in1=mean[:],
                                    op=AluOp.mult)
            var = sb1.tile([1, OUT], F32, tag="var")
            nc.vector.tensor_tensor(out=var[:], in0=ev2[:], in1=m2[:],
                                    op=AluOp.subtract)
            nc.vector.tensor_scalar(var[:], var[:], EPS, None, AluOp.add)
            std = sb1.tile([1, OUT], F32, tag="std")
            nc.scalar.sqrt(std[:], var[:])
            rstd = sb1.tile([1, OUT], F32, tag="rstd")
            nc.vector.reciprocal(rstd[:], std[:])
            bng = sb1.tile([1, OUT], F32, tag="bng")
            bnb = sb1.tile([1, OUT], F32, tag="bnb")
            if last:
                nc.sync.dma_start(out=bng[:], in_=ins["bn_g_l"][:])
                nc.sync.dma_start(out=bnb[:], in_=ins["bn_b_l"][:])
            else:
                nc.sync.dma_start(out=bng[:], in_=ins["bn_g"][li])
                nc.sync.dma_start(out=bnb[:], in_=ins["bn_b"][li])
            sg = sb1.tile([1, OUT], F32, tag="sg")
            nc.vector.tensor_tensor(out=sg[:], in0=rstd[:], in1=bng[:],
                                    op=AluOp.mult)
            c0 = sb1.tile([1, OUT], F32, tag="c0")
            nc.vector.tensor_tensor(out=c0[:], in0=mean[:], in1=sg[:],
                                    op=AluOp.mult)
            crow = sb1.tile([1, OUT], F32, tag="crow")
            nc.vector.tensor_tensor(out=crow[:], in0=bnb[:], in1=c0[:],
                                    op=AluOp.subtract)
            reps = []
            for rsrc in (sg, crow):
                rp = ps.tile([128, OUT], F32, tag="ps")
                nc.tensor.matmul(out=rp[:], lhsT=onesrow[:], rhs=rsrc[:],
                                 start=True, stop=True)
                rt = sb1.tile([128, OUT], F32, tag=f"rep{len(reps)}")
                nc.scalar.copy(out=rt[:], in_=rp[:])
                reps.append(rt)

            def rep_b(rt):
                return (rt[:].rearrange("p (o c) -> p o c", o=1)
                        .broadcast_to([128, NG, OUT]))

            bn = sq  # reuse buffer
            aggv = agg[:].rearrange("p (g c) -> p g c", c=OUT)
            bnv = bn[:].rearrange("p (g c) -> p g c", c=OUT)
            nc.vector.tensor_tensor(out=bnv, in0=aggv, in1=rep_b(reps[0]),
                                    op=AluOp.mult)
            nc.vector.tensor_tensor(out=bnv, in0=bnv, in1=rep_b(reps[1]),
                                    op=AluOp.add)
            nc.vector.tensor_scalar(bn[:], bn[:], 0.0, None, AluOp.max)

            if last:
                nc.sync.dma_start(out=outs["out"][:], in_=bn[:])
            else:
                h_new = sb.tile([128, NG * HID], F32, tag="h")
                nc.vector.tensor_tensor(out=h_new[:], in0=bn[:], in1=h_cur[:],
                                        op=AluOp.add)
                h_cur = h_new
                push_table(h_cur[:])

    for _rep in range(NREPEAT):
        one_forward()

    stack.close()


# ---------------------------------------------------------------------------
# top-level entry
# ---------------------------------------------------------------------------

def _make_in_maps(g, weights):
    in_maps = []
    for c in range(NCORES):
        pc = g["per_core"][c]
        m = dict(weights)
        m["featT"] = g["featT"][c]
        m["ident"] = np.eye(g["hid"], dtype=np.float32)
        m["idx_ev"] = pc["idx_ev"]
        m["idx_od"] = pc["idx_od"]
        m["eq"] = pc["eq"]
        m["dr"] = pc["dr"]
        m["dc"] = pc["dc"]
        in_maps.append({k + "_d": v for k, v in m.items()})
    return in_maps


def _weights_dict(inputs, g):
    f32 = lambda x: np.ascontiguousarray(np.asarray(x, np.float32))
    bf16 = lambda x: np.ascontiguousarray(np.asarray(x, ml_dtypes.bfloat16))
    nhl, k, hid, ncls = g["nhl"], g["k"], g["hid"], g["ncls"]
    fc_wb = np.concatenate(
        [np.asarray(inputs["fc_w"], np.float32),
         np.asarray(inputs["fc_b"], np.float32).reshape(nhl, 1, k * hid)],
        axis=1)                                      # [nhl, 65, k*hid]
    fc_wb_l = np.concatenate(
        [np.asarray(inputs["fc_w_l"], np.float32),
         np.asarray(inputs["fc_b_l"], np.float32).reshape(1, k * ncls)],
        axis=0)                                      # [65, k*ncls]
    w = dict(
        emb_w=f32(inputs["emb_w"]),                  # [128, 64]
        emb_b=f32(inputs["emb_b"]).reshape(1, -1),
        fc_wb=f32(fc_wb),
        fc_wb_l=f32(fc_wb_l),
        mu=f32(inputs["mu"]).reshape(nhl, 1, -1),
        inv_sigma=f32(inputs["inv_sigma"]).reshape(nhl, 1, -1),
        pp_w=f32(inputs["pp_w"]).reshape(nhl, 1, -1),
        pp_b=f32(inputs["pp_b"]).reshape(nhl, 1, -1),
        bn_g=f32(inputs["bn_g"]).reshape(nhl, 1, -1),
        bn_b=f32(inputs["bn_b"]).reshape(nhl, 1, -1),
        mu_l=f32(inputs["mu_l"]).reshape(1, -1),
        inv_sigma_l=f32(inputs["inv_sigma_l"]).reshape(1, -1),
        pp_w_l=f32(inputs["pp_w_l"]).reshape(1, -1),
        pp_b_l=f32(inputs["pp_b_l"]).reshape(1, -1),
        bn_g_l=f32(inputs["bn_g_l"]).reshape(1, -1),
        bn_b_l=f32(inputs["bn_b_l"]).reshape(1, -1),
    )
    return w


def _build_featT(inputs, g):
    feat = np.asarray(inputs["feature"], np.float32)
    featT = []
    for c in range(NCORES):
        arr = np.zeros((g["in_dim"], g["npc"]), np.float32)
        nds = np.flatnonzero(g["core_of"] == c)
        arr[:, g["slot_of"][nds] % g["npc"]] = feat[nds].T
        featT.append(arr)
    g["featT"] = featT


def run_device(g, weights, trace=False):
    nc = bacc.Bacc("TRN2", target_bir_lowering=False, debug=False,
                   num_devices=NCORES, num_swdge_queues=4)
    ins_ap, outs_ap = {}, {}
    in_maps = _make_in_maps(g, weights)
    for name, arr in in_maps[0].items():
        t = nc.dram_tensor(name, list(arr.shape), mybir.dt.from_np(arr.dtype),
                           kind="ExternalInput")
        ins_ap[name[:-2]] = t.ap()
    out_t = nc.dram_tensor("out_d", [128, g["NG"] * g["ncls"]], F32,
                           kind="ExternalOutput")
    outs_ap["out"] = out_t.ap()

    with tile.TileContext(nc) as tc:
        build(tc, outs_ap, ins_ap, g)
    nc.compile()

    res = bass_utils.run_bass_kernel_spmd(
        nc, in_maps, core_ids=list(range(NCORES)), trace=trace)
    return res


def assemble_output(g, res):
    out = np.zeros((g["n"], g["ncls"]), np.float32)
    for c in range(NCORES):
        oc = res.results[c]["out_d"].reshape(128, g["NG"], g["ncls"])
        nds = np.flatnonzero(g["core_of"] == c)
        sl = g["slot_of"][nds] % g["npc"]
        out[nds] = oc[sl % 128, sl // 128, :]
    return out


def kernel(**inputs):
    g = preprocess(np.asarray(inputs["edge_index"]), GEOM_REAL)
    _build_featT(inputs, g)
    weights = _weights_dict(inputs, g)
    res = run_device(g, weights, trace=os.environ.get("MONET_TRACE", "0") == "1")
    out = assemble_output(g, res)
    kernel.last_exec_time_ns = getattr(res, "exec_time_ns", None)
    return out


# ---------------------------------------------------------------------------
# timed execution (repeated PJRT calls on a single compiled executable)
# ---------------------------------------------------------------------------

def run_device_timed(g, weights, n_iters=5):
    import time
    import jax
    from jax.sharding import Mesh, PartitionSpec
    from jax.experimental.shard_map import shard_map
    from concourse import bass2jax as b2j

    nc = bacc.Bacc("TRN2", target_bir_lowering=False, debug=False,
                   num_devices=NCORES, num_swdge_queues=4)
    ins_ap = {}
    in_maps = _make_in_maps(g, weights)
    for name, arr in in_maps[0].items():
        t = nc.dram_tensor(name, list(arr.shape), mybir.dt.from_np(arr.dtype),
                           kind="ExternalInput")
        ins_ap[name[:-2]] = t.ap()
    out_t = nc.dram_tensor("out_d", [128, g["NG"] * g["ncls"]], F32,
                           kind="ExternalOutput")
    outs_ap = {"out": out_t.ap()}
    with tile.TileContext(nc) as tc:
        build(tc, outs_ap, ins_ap, g)
    nc.compile()

    b2j.install_neuronx_cc_hook()
    partition_name = (nc.partition_id_tensor.name
                      if nc.partition_id_tensor else None)
    in_names, out_names, out_avals, zero_outs = [], [], [], []
    for alloc in nc.m.functions[0].allocations:
        if not isinstance(alloc, mybir.MemoryLocationSet):
            continue
        name = alloc.memorylocations[0].name
        if alloc.kind == "ExternalInput":
            if name != partition_name:
                in_names.append(name)
        elif alloc.kind == "ExternalOutput":
            dt = mybir.dt.np(alloc.dtype)
            out_avals.append(jax.core.ShapedArray(tuple(alloc.tensor_shape), dt))
            out_names.append(name)
            zero_outs.append(np.zeros(tuple(alloc.tensor_shape), dt))
    n_params = len(in_names)
    n_outs = len(out_names)
    in_names = in_names + out_names
    if partition_name is not None:
        in_names.append(partition_name)
    donate = tuple(range(n_params, n_params + n_outs))

    def _body(*args):
        operands = list(args)
        if partition_name is not None:
            operands.append(b2j.partition_id_tensor())
        outs = b2j._bass_exec_p.bind(
            *operands,
            out_avals=tuple(out_avals),
            in_names=tuple(in_names),
            out_names=tuple(out_names),
            lowering_input_output_aliases=(),
            sim_require_finite=True,
            sim_require_nnan=True,
            nc=nc,
        )
        return tuple(outs)

    devices = jax.devices()[:NCORES]
    mesh = Mesh(np.asarray(devices), ("core",))
    sharded = jax.jit(
        shard_map(_body, mesh=mesh,
                  in_specs=(PartitionSpec("core"),) * (n_params + n_outs),
                  out_specs=(PartitionSpec("core"),) * n_outs,
                  check_rep=False),
        donate_argnums=donate, keep_unused=True)
    per_core = [[np.asarray(m[nm]) for nm in in_names[:n_params]]
                for m in in_maps]
    concat_in = [np.concatenate([per_core[c][i] for c in range(NCORES)], 0)
                 for i in range(n_params)]
    concat_in = [jax.device_put(a) for a in concat_in]

    times = []
    out_arrs = None
    for it in range(n_iters):
        czeros = [np.zeros((NCORES * z.shape[0], *z.shape[1:]), z.dtype)
                  for z in zero_outs]
        t0 = time.perf_counter()
        out_arrs = sharded(*concat_in, *czeros)
        jax.block_until_ready(out_arrs)
        times.append(time.perf_counter() - t0)
    results = [
        {nm: np.asarray(out_arrs[i]).reshape(NCORES, *out_avals[i].shape)[c]
         for i, nm in enumerate(out_names)}
        for c in range(NCORES)
    ]

    class R:
        pass
    r = R()
    r.results = results
    r.exec_time_ns = int(min(times[1:]) * 1e9) if len(times) > 1 else None
    r.all_times = times
    return r


# revision 8
# speedup vs baseline: 46.8367x; 1.1981x over previous
"""MoNet (GMM graph conv) on Trainium2 — 8-core SPMD Bass/Tile kernel.

Sharding: dst-node slices per core (edge-parallel within core), with node
relabeling into per-core "slot space". Uniform SPMD program; per-core data.

v2 layout: gapless edge tiles. Per (chunk, window, class) a GLOBAL capacity
(max edge count over cores) reserves a contiguous position range; positions
pack into 128-wide gather tiles that may straddle window boundaries. A
(tile, window) pair is a "segment": the PE matmul for window w over a
boundary tile uses an s3 block whose other-window partitions are zero.
 - gather: dma_gather 256B rows (64 bf16 feats + ones col + pad) via two
   stride-2 table views (int16 idx range), indices sorted within each
   (window, class) run for HBM locality.
 - s3 = one-hot(dst slot) x gauss built by DVE from uint8 one-hot.
 - PSUM: 4 windows share one bank [65, 4, K*32]; one Act copy stages all 4
   to bf16 ust; fc matmuls in bf16.
 - BN stats via DVE group-reduce + 1-col PE matmuls + AllReduce; h pushed
   compact (64 cols bf16) through a Shared-output AllGather into the
   256B-row gather table (ones column initialized once).
"""
import sys, os
import numpy as np

if "/opt/trn_rl_repo" not in sys.path:
    sys.path.insert(0, "/opt/trn_rl_repo")

import ml_dtypes
from concourse import bass, bacc, mybir, tile
from concourse import bass_utils

AluOp = mybir.AluOpType
Act = mybir.ActivationFunctionType
F32 = mybir.dt.float32
BF16 = mybir.dt.bfloat16
I16 = mybir.dt.int16
U8 = mybir.dt.uint8

NCORES = 8
EPS = 1e-5

GEOM_REAL = dict(n=50000, e=800000, in_dim=128, hid=64, k=3, pdim=2,
                 ncls=16, nhl=3, W=208, wpchunk=16)


def derive(geom):
    g = dict(geom)
    g["npc"] = g["W"] * 32                # dst slots per core
    g["NG"] = g["npc"] // 128             # 128-slot groups per core
    g["NCH"] = g["W"] // g["wpchunk"]     # chunks per layer
    g["n_rows"] = NCORES * g["npc"]       # table rows
    assert g["n_rows"] // 2 <= 32767
    return g


# ---------------------------------------------------------------------------
# host preprocessing (pure integer/index manipulation)
# ---------------------------------------------------------------------------

def preprocess(edge_index, geom):
    g = derive(geom)
    n, W, npc = g["n"], g["W"], g["npc"]
    NCH, wpc = g["NCH"], g["wpchunk"]
    row = np.asarray(edge_index[0], np.int64)
    col = np.asarray(edge_index[1], np.int64)
    deg_r = np.bincount(row, minlength=n).astype(np.int64)
    deg_c = np.bincount(col, minlength=n).astype(np.int64)

    # 1) nodes -> cores (snake deal by in-degree for balanced edge counts)
    order = np.argsort(-deg_c, kind="stable")
    core_of = np.empty(n, np.int64)
    blk = np.arange(n) // NCORES
    pos = np.arange(n) % NCORES
    snake = np.where(blk % 2 == 0, pos, NCORES - 1 - pos)
    core_of[order] = snake

    # 2) class A (even table rows) = per-core top half by out-degree
    is_a = np.zeros(n, bool)
    for c in range(NCORES):
        nds = np.flatnonzero(core_of == c)
        half = min((len(nds) + 1) // 2, W * 16)
        topa = nds[np.argsort(-deg_r[nds], kind="stable")][:half]
        is_a[topa] = True

    src_a = is_a[row]
    in_ev = np.bincount(col[src_a], minlength=n).astype(np.int64)
    in_od = np.bincount(col[~src_a], minlength=n).astype(np.int64)

    # 3) per-core window packing (first-fit decreasing, loose caps only to
    #    balance chunks) then window relabel by total load so that per-rank
    #    counts align across cores (tight global caps).
    cap_ev, cap_od = 6 * 128, 4 * 128
    slot_of = np.full(n, -1, np.int64)
    wcnt_ev = np.zeros((NCORES, W), np.int64)
    wcnt_od = np.zeros((NCORES, W), np.int64)
    for c in range(NCORES):
        nds = np.flatnonzero(core_of == c)
        nds = nds[np.argsort(-(in_ev[nds] + in_od[nds]), kind="stable")]
        wev = np.zeros(W, np.int64); wod = np.zeros(W, np.int64)
        wna = np.zeros(W, np.int64); wnb = np.zeros(W, np.int64)
        wslot = np.full(n, -1, np.int64)
        for nd in nds:
            a = bool(is_a[nd])
            for w in range(W):
                if a and wna[w] >= 16: continue
                if (not a) and wnb[w] >= 16: continue
                if wev[w] + in_ev[nd] > cap_ev: continue
                if wod[w] + in_od[nd] > cap_od: continue
                if a:
                    j = 2 * wna[w]; wna[w] += 1
                else:
                    j = 2 * wnb[w] + 1; wnb[w] += 1
                wev[w] += in_ev[nd]; wod[w] += in_od[nd]
                wslot[nd] = w * 32 + j
                break
            else:
                raise RuntimeError(f"window packing failed (core {c})")
        # relabel windows: sort by (ev+od) load descending so rank k has the
        # k-th largest load on every core -> per-rank max over cores ~= mean
        perm = np.argsort(-(wev + wod), kind="stable")   # rank -> old w
        rank_of = np.empty(W, np.int64)
        # deal ranks round-robin over chunks: rank r -> window
        # (r % NCH)*wpc + r//NCH, so every chunk gets a balanced mix and
        # per-rank loads still align across cores
        rr = (np.arange(W) % NCH) * wpc + np.arange(W) // NCH
        rank_of[perm] = rr
        sel = wslot >= 0
        slot_of[sel] = (core_of[sel] * npc + rank_of[wslot[sel] // 32] * 32
                        + wslot[sel] % 32)
        wcnt_ev[c][rr] = wev[perm]
        wcnt_od[c][rr] = wod[perm]

    assert (slot_of >= 0).all()
    assert (slot_of[is_a] % 2 == 0).all() and (slot_of[~is_a] % 2 == 1).all()

    g.update(core_of=core_of, slot_of=slot_of, deg_r=deg_r, deg_c=deg_c)

    # 4) global segment template from per-(window, class) caps
    caps_ev = wcnt_ev.max(axis=0)        # [W]
    caps_od = wcnt_od.max(axis=0)
    EVN = np.zeros(NCH, np.int64)        # gather idx count per chunk (even)
    ODN = np.zeros(NCH, np.int64)
    NSEG = np.zeros(NCH, np.int64)
    seg_template = []                    # per ch: dict keyed (wl, cls, tile)->col
    base_ev = np.zeros(W, np.int64)      # position base of window's run
    base_od = np.zeros(W, np.int64)
    win_segs = []                        # per ch: list over wl of [(cls, tile, col)]
    for ch in range(NCH):
        ws = range(ch * wpc, (ch + 1) * wpc)
        segcol = {}
        wl_segs = [[] for _ in range(wpc)]
        for cls, caps, base_arr, tot in ((0, caps_ev, base_ev, None),
                                         (1, caps_od, base_od, None)):
            p0 = 0
            for wl, w in enumerate(ws):
                base_arr[w] = p0
                cap = int(caps[w])
                if cap > 0:
                    for t in range(p0 // 128, (p0 + cap - 1) // 128 + 1):
                        key = (wl, cls, t)
                        if key not in segcol:
                            segcol[key] = -1   # placeholder, ordered later
                        wl_segs[wl].append((cls, t))
                p0 += cap
            if cls == 0:
                EVN[ch] = -(-p0 // 128) * 128
            else:
                ODN[ch] = -(-p0 // 128) * 128
        # assign columns ordered by (wl, cls, tile)
        cols = 0
        for wl in range(wpc):
            newl = []
            for cls, t in wl_segs[wl]:
                segcol[(wl, cls, t)] = cols
                newl.append((cls, t, cols))
                cols += 1
            wl_segs[wl] = newl
        NSEG[ch] = cols
        seg_template.append(segcol)
        win_segs.append(wl_segs)

    SEG_CAP = int(NSEG.max())
    EVT_CAP = int((EVN // 128).max())
    ODT_CAP = int((ODN // 128).max())
    g.update(EVN=EVN.tolist(), ODN=ODN.tolist(), NSEG=NSEG.tolist(),
             SEG_CAP=SEG_CAP, EVT_CAP=EVT_CAP, ODT_CAP=ODT_CAP,
             win_segs=win_segs, SEGT=NCH * SEG_CAP)
    tot_slots = int(EVN.sum() + ODN.sum())
    g["gather_slots"] = tot_slots

    # 5) per-core arrays
    e_core = core_of[col]
    e_slot = slot_of[col] % npc
    e_w = e_slot // 32
    e_j = e_slot % 32
    e_view = (slot_of[row] // 2).astype(np.int64)   # stride-2 view index

    per_core = []
    for c in range(NCORES):
        idx_ev = np.zeros((NCH, 16, EVT_CAP * 8), np.int16)
        idx_od = np.zeros((NCH, 16, ODT_CAP * 8), np.int16)
        eq = np.zeros((NCH, 128, SEG_CAP * 32), np.uint8)
        dr = np.zeros((128, NCH * SEG_CAP), np.float32)
        dc = np.zeros((128, NCH * SEG_CAP), np.float32)

        sel = np.flatnonzero(e_core == c)
        ew, ej, ecls = e_w[sel], e_j[sel], src_a[sel]
        evi = e_view[sel]
        edr = deg_r[row[sel]].astype(np.float32)
        edc = deg_c[col[sel]].astype(np.float32)
        # group edges by (window, class); sort by view index inside a group
        okey = ew * 2 + (~ecls).astype(np.int64)
        eorder = np.lexsort((evi, okey))
        bnd = np.searchsorted(okey[eorder], np.arange(2 * W + 1))
        for w in range(W):
            ch, wl = divmod(w, wpc)
            segcol = seg_template[ch]
            for cls in (0, 1):
                kk = w * 2 + cls
                eids = eorder[bnd[kk]:bnd[kk + 1]]
                ne = len(eids)
                if ne == 0:
                    continue
                b = base_ev[w] if cls == 0 else base_od[w]
                q = np.arange(ne)
                pp = (b + q) % 128
                tt = (b + q) // 128
                scol = np.array([segcol[(wl, cls, int(t))] for t in tt])
                tgt = idx_ev if cls == 0 else idx_od
                tgt[ch][(b + q) % 16, (b + q) // 16] = evi[eids].astype(np.int16)
                eq[ch][pp, scol * 32 + ej[eids]] = 1
                dr[pp, ch * SEG_CAP + scol] = edr[eids]
                dc[pp, ch * SEG_CAP + scol] = edc[eids]
        idx_ev = np.tile(idx_ev, (1, 8, 1))
        idx_od = np.tile(idx_od, (1, 8, 1))
        per_core.append(dict(idx_ev=idx_ev, idx_od=idx_od, eq=eq, dr=dr, dc=dc))
    g["per_core"] = per_core
    return g


# ---------------------------------------------------------------------------
# device program
# ---------------------------------------------------------------------------

def build(tc, outs, ins, g):
    nc = tc.nc
    W, npc, NG = g["W"], g["npc"], g["NG"]
    NCH, wpc = g["NCH"], g["wpchunk"]
    HID, KK, NCLS, NHL = g["hid"], g["k"], g["ncls"], g["nhl"]
    n_rows = g["n_rows"]
    nn = g["n"]
    SEG_CAP, SEGT = g["SEG_CAP"], g["SEGT"]
    EVT_CAP, ODT_CAP = g["EVT_CAP"], g["ODT_CAP"]
    EVN, ODN, NSEG = g["EVN"], g["ODN"], g["NSEG"]
    win_segs = g["win_segs"]

    import contextlib
    stack = contextlib.ExitStack()
    sbc = stack.enter_context(tc.tile_pool(name="sbc", bufs=1))
    sb1 = stack.enter_context(tc.tile_pool(name="sb1", bufs=1))
    sb = stack.enter_context(tc.tile_pool(name="sb", bufs=2))
    ps = stack.enter_context(tc.tile_pool(name="ps", bufs=8, space="PSUM"))
    dram = stack.enter_context(tc.tile_pool(name="dram", bufs=1, space="DRAM"))

    # ---- constants / persistent state
    onesrow = sbc.tile([1, 128], F32); nc.vector.memset(onesrow[:], 1.0)
    onescol = sbc.tile([128, 1], F32); nc.vector.memset(onescol[:], 1.0)
    ident = sbc.tile([HID, HID], F32)
    nc.sync.dma_start(out=ident[:], in_=ins["ident"][:])
    stage = sbc.tile([128, NG, 128], BF16)
    nc.vector.memset(stage[:], 0.0)
    nc.vector.memset(stage[:, :, 64:65], 1.0)
    srcs = sbc.tile([128, SEGT], F32)
    dsts = sbc.tile([128, SEGT], F32)
    gauss = sbc.tile([128, KK, SEGT], BF16)

    table = dram.tile([n_rows, 128], BF16)
    stage_d = dram.tile([npc, 128], BF16)
    stats_in = dram.tile([HID, 2], F32)
    stats_out = dram.tile([HID, 2], F32)

    zz = sbc.tile([HID, 2], F32)
    nc.vector.memset(zz[:], 0.0)
    nc.sync.dma_start(out=stats_in[:], in_=zz[:])
    nc.sync.dma_start(out=stats_out[:], in_=zz[:])

    tbl_ev = table[:].rearrange("(m two) c -> m (two c)", two=2)[:, 0:128]
    tbl_od = table[:].rearrange("(m two) c -> m (two c)", two=2)[:, 128:256]

    # ---- prologue: pseudo coords
    with tc.tile_pool(name="pro", bufs=1) as pro:
        drt = pro.tile([128, SEGT], F32)
        nc.sync.dma_start(out=drt[:], in_=ins["dr"][:])
        dct = pro.tile([128, SEGT], F32)
        nc.sync.dma_start(out=dct[:], in_=ins["dc"][:])
        t0 = pro.tile([128, SEGT], F32)
        for dsrc, dout in ((drt, srcs), (dct, dsts)):
            nc.vector.tensor_scalar(t0[:], dsrc[:], 1.0, None, AluOp.add)
            nc.scalar.sqrt(t0[:], t0[:])
            nc.vector.reciprocal(dout[:], t0[:])

    NO_CC = os.environ.get("MONET_NO_CC", "0") == "1"
    NHID_RUN = int(os.environ.get("MONET_NLAYERS", str(NHL)))
    NREPEAT = int(os.environ.get("MONET_REPEAT", "1"))

    def push_table(h_flat):
        # h_flat [128, NG*64] f32 -> bf16 stage (+ones col) -> AllGather
        nc.vector.tensor_copy(
            out=stage[:, :, 0:64],
            in_=h_flat.rearrange("p (g c) -> p g c", c=64))
        nc.sync.dma_start(
            out=stage_d[:].rearrange("(gp p) c -> p gp c", p=128),
            in_=stage[:])
        if NO_CC:
            nc.sync.dma_start(out=table[0:npc, :], in_=stage_d[:])
            return
        nc.gpsimd.collective_compute(
            "AllGather", AluOp.bypass, replica_groups=[list(range(NCORES))],
            ins=[stage_d[:].opt()], outs=[table[:].opt()])

    def one_forward():
        # ---- embed: h0 = featT.T @ emb_w + emb_b
        h_cur = sb.tile([128, NG * HID], F32, tag="h")
        with tc.tile_pool(name="emb", bufs=2) as emb:
            embw = emb.tile([128, HID], F32, tag="embw")
            nc.sync.dma_start(out=embw[:], in_=ins["emb_w"][:])
            ebrow = emb.tile([1, HID], F32, tag="ebrow")
            nc.sync.dma_start(out=ebrow[:], in_=ins["emb_b"][:])
            for gi in range(NG):
                ft = emb.tile([128, 128], F32, tag="ft")
                nc.sync.dma_start(out=ft[:],
                                  in_=ins["featT"][:, gi * 128:(gi + 1) * 128])
                ep = ps.tile([128, HID], F32, tag="ps")
                nc.tensor.matmul(out=ep[:], lhsT=ft[:],
                                 rhs=embw[:], start=True, stop=True)
                nc.scalar.copy(out=h_cur[:, gi * HID:(gi + 1) * HID], in_=ep[:])
            ebp = ps.tile([128, HID], F32, tag="ps")
            nc.tensor.matmul(out=ebp[:], lhsT=onesrow[:], rhs=ebrow[:],
                             start=True, stop=True)
            ebrep = emb.tile([128, HID], F32)
            nc.scalar.copy(out=ebrep[:], in_=ebp[:])
            nc.vector.tensor_tensor(
                out=h_cur[:], in0=h_cur[:],
                in1=ebrep[:].rearrange("p (o c) -> p o c", o=1)
                    .broadcast_to([128, NG, HID]),
                op=AluOp.add)
        push_table(h_cur[:])

        # ---- layers
        for li in list(range(NHID_RUN)) + [NHL]:
            last = li == NHL
            OUT = NCLS if last else HID

            # scalars row: [w00 w01 w10 w11 b0 b1 | mu k*2+d | isg k*2+d]
            scal_row = sb1.tile([1, 32], F32, tag="scalrow")
            nc.vector.memset(scal_row[:], 0.0)
            if last:
                nc.sync.dma_start(out=scal_row[:, 0:4], in_=ins["pp_w_l"][:])
                nc.sync.dma_start(out=scal_row[:, 4:6], in_=ins["pp_b_l"][:])
                nc.sync.dma_start(out=scal_row[:, 6:6 + 2 * KK], in_=ins["mu_l"][:])
                nc.sync.dma_start(out=scal_row[:, 18:18 + 2 * KK],
                                  in_=ins["inv_sigma_l"][:])
            else:
                nc.sync.dma_start(out=scal_row[:, 0:4], in_=ins["pp_w"][li])
                nc.sync.dma_start(out=scal_row[:, 4:6], in_=ins["pp_b"][li])
                nc.sync.dma_start(out=scal_row[:, 6:6 + 2 * KK], in_=ins["mu"][li])
                nc.sync.dma_start(out=scal_row[:, 18:18 + 2 * KK],
                                  in_=ins["inv_sigma"][li])
            scp = ps.tile([128, 32], F32, tag="ps")
            nc.tensor.matmul(out=scp[:], lhsT=onesrow[:], rhs=scal_row[:],
                             start=True, stop=True)
            scal = sb1.tile([128, 32], F32, tag="scal")
            nc.scalar.copy(out=scal[:], in_=scp[:])

            def sc(j):
                return scal[:, j:j + 1]

            # gauss[k] = exp(-0.5*(((ps0-mu_k0)*is_k0)^2+((ps1-mu_k1)*is_k1)^2))
            ps0 = sb1.tile([128, SEGT], F32, tag="ps0")
            ps1 = sb1.tile([128, SEGT], F32, tag="ps1")
            ta = sb1.tile([128, SEGT], F32, tag="ta")
            tb = sb1.tile([128, SEGT], F32, tag="tb")
            for (pst, wA, wB, bB) in ((ps0, 0, 2, 4), (ps1, 1, 3, 5)):
                nc.vector.tensor_scalar(ta[:], srcs[:], sc(wA), None, AluOp.mult)
                nc.vector.tensor_scalar(tb[:], dsts[:], sc(wB), None, AluOp.mult)
                nc.vector.tensor_tensor(out=ta[:], in0=ta[:], in1=tb[:],
                                        op=AluOp.add)
                nc.scalar.activation(pst[:], ta[:], Act.Tanh, bias=sc(bB),
                                     scale=1.0)
            for k in range(KK):
                nc.vector.tensor_scalar(ta[:], ps0[:], sc(6 + 2 * k),
                                        sc(18 + 2 * k),
                                        AluOp.subtract, AluOp.mult)
                nc.vector.tensor_scalar(tb[:], ps1[:], sc(7 + 2 * k),
                                        sc(19 + 2 * k),
                                        AluOp.subtract, AluOp.mult)
                nc.scalar.square(ta[:], ta[:])
                nc.scalar.square(tb[:], tb[:])
                nc.vector.tensor_tensor(out=ta[:], in0=ta[:], in1=tb[:],
                                        op=AluOp.add)
                nc.scalar.activation(gauss[:, k, :], ta[:], Act.Exp,
                                     bias=0.0, scale=-0.5)

            # dense weights [65, K*OUT] bf16 (stacked [w; b] on host)
            fcwb = sb1.tile([65, KK * OUT], F32, tag="fcwb")
            if last:
                nc.sync.dma_start(out=fcwb[:], in_=ins["fc_wb_l"][:])
            else:
                nc.sync.dma_start(out=fcwb[:], in_=ins["fc_wb"][li])

            agg = sb1.tile([128, NG * OUT], F32, tag="aggsb")

            # ---- edge pipeline
            for ch in range(NCH):
                nev_t, nod_t = EVN[ch] // 128, ODN[ch] // 128
                nseg = NSEG[ch]
                FULLG = os.environ.get("MONET_FULLG", "0") == "1"
                evn = EVT_CAP * 128 if FULLG else EVN[ch]
                odn = ODT_CAP * 128 if FULLG else ODN[ch]
                iev = sb.tile([128, EVT_CAP * 8], I16, tag="iev")
                nc.sync.dma_start(out=iev[:, 0:evn // 16],
                                  in_=ins["idx_ev"][ch][:, 0:evn // 16])
                iod = sb.tile([128, ODT_CAP * 8], I16, tag="iod")
                nc.sync.dma_start(out=iod[:, 0:odn // 16],
                                  in_=ins["idx_od"][ch][:, 0:odn // 16])
                eqt = sb.tile([128, SEG_CAP, 32], U8, tag="eq")
                nc.sync.dma_start(
                    out=eqt[:].rearrange("p s j -> p (s j)")[:, 0:nseg * 32],
                    in_=ins["eq"][ch][:, 0:nseg * 32])
                hg_lo = sb.tile([128, EVT_CAP, 128], BF16, tag="hglo")
                hg_hi = sb.tile([128, ODT_CAP, 128], BF16, tag="hghi")
                if os.environ.get("MONET_NO_GATHER", "0") == "1":
                    nc.vector.memset(hg_lo[:], 0.5)
                    nc.vector.memset(hg_hi[:], 0.5)
                else:
                    nc.gpsimd.dma_gather(
                        out_ap=hg_lo[:, 0:evn // 128, :], in_ap=tbl_ev,
                        idxs_ap=iev[:, 0:evn // 16],
                        num_idxs=evn, num_idxs_reg=evn,
                        elem_size=128, elem_step=256,
                        single_packet=os.environ.get("MONET_SP", "0") == "1",
                        queue_num=ch % 4)
                    nc.gpsimd.dma_gather(
                        out_ap=hg_hi[:, 0:odn // 128, :], in_ap=tbl_od,
                        idxs_ap=iod[:, 0:odn // 16],
                        num_idxs=odn, num_idxs_reg=odn,
                        elem_size=128, elem_step=256,
                        single_packet=os.environ.get("MONET_SP", "0") == "1",
                        queue_num=(ch + 2) % 4)
                s3 = sb.tile([128, KK, SEG_CAP, 32], BF16, tag="s3")
                for k in range(KK):
                    nc.vector.tensor_tensor(
                        out=s3[:, k, 0:nseg, :], in0=eqt[:, 0:nseg, :],
                        in1=gauss[:, k, ch * SEG_CAP:ch * SEG_CAP + nseg]
                            .rearrange("p (s o) -> p s o", o=1)
                            .broadcast_to([128, nseg, 32]),
                        op=AluOp.mult)
                OLDPSUM = os.environ.get("MONET_OLDPSUM", "0") == "1"
                for wl in range(wpc):
                    sub = wl % 4
                    if OLDPSUM:
                        win = ps.tile([65, KK * 32], F32, tag="ps")
                    elif sub == 0:
                        winq = ps.tile([65, 4, KK * 32], F32, tag="ps")
                    segs = win_segs[ch][wl]
                    if not segs:
                        nc.vector.memset(
                            (win[:] if OLDPSUM else winq[:, sub, :]), 0.0)
                    for si, (cls, t, scol) in enumerate(segs):
                        lhs = (hg_lo if cls == 0 else hg_hi)[:, t, 0:65]
                        nc.tensor.matmul(
                            out=(win[:] if OLDPSUM else winq[:, sub, :]),
                            lhsT=lhs,
                            rhs=s3[:, :, scol, :],
                            start=(si == 0), stop=(si == len(segs) - 1))
                    if OLDPSUM:
                        if sub == 0:
                            ust = sb.tile([65, KK, 4, 32], F32, tag="ust")
                        nc.scalar.copy(
                            out=ust[:, :, sub, :],
                            in_=win[:].rearrange("u (k j) -> u k j", j=32))
                    if sub == 3:
                        gi = (ch * wpc + wl) // 4
                        if not OLDPSUM:
                            ust = sb.tile([65, KK, 4, 32], F32, tag="ust")
                            nc.vector.tensor_copy(
                                out=ust[:].rearrange("u k s j -> u s k j"),
                                in_=winq[:].rearrange("u s (k j) -> u s k j",
                                                      j=32))
                        ap_ = ps.tile([128, OUT], F32, tag="ps")
                        for k in range(KK):
                            lhsu = ust[:, k].rearrange("u s j -> u (s j)")
                            nc.tensor.matmul(
                                out=ap_[:], lhsT=lhsu,
                                rhs=fcwb[:, k * OUT:(k + 1) * OUT],
                                start=(k == 0), stop=(k == KK - 1))
                        nc.scalar.copy(out=agg[:, gi * OUT:(gi + 1) * OUT],
                                       in_=ap_[:])

            # ---- BN stats: group-reduce on DVE, cross-partition via 1-col PE
            sq = sb1.tile([128, NG * OUT], F32, tag="sq")
            nc.scalar.square(sq[:], agg[:])
            aggr = sb1.tile([128, OUT], F32, tag="aggr")
            nc.vector.tensor_reduce(
                out=aggr[:], in_=agg[:].rearrange("p (g c) -> p c g", c=OUT),
                axis=mybir.AxisListType.X, op=AluOp.add)
            sqr = sb1.tile([128, OUT], F32, tag="sqr")
            nc.vector.tensor_reduce(
                out=sqr[:], in_=sq[:].rearrange("p (g c) -> p c g", c=OUT),
                axis=mybir.AxisListType.X, op=AluOp.add)
            sump = ps.tile([OUT, 1], F32, tag="ps")
            nc.tensor.matmul(out=sump[:], lhsT=aggr[:], rhs=onescol[:],
                             start=True, stop=True)
            sqp = ps.tile([OUT, 1], F32, tag="ps")
            nc.tensor.matmul(out=sqp[:], lhsT=sqr[:], rhs=onescol[:],
                             start=True, stop=True)
            stats = sb1.tile([OUT, 2], F32, tag="stats")
            nc.scalar.copy(out=stats[:, 0:1], in_=sump[:])
            nc.scalar.copy(out=stats[:, 1:2], in_=sqp[:])
            nc.sync.dma_start(out=stats_in[0:OUT, :], in_=stats[:])
            if NO_CC:
                nc.sync.dma_start(out=stats_out[0:OUT, :], in_=stats_in[0:OUT, :])
            else:
                nc.gpsimd.collective_compute(
                    "AllReduce", AluOp.add,
                    replica_groups=[list(range(NCORES))],
                    ins=[stats_in[:].opt()], outs=[stats_out[:].opt()])
            stats_ar = sb1.tile([OUT, 2], F32, tag="statsar")
            nc.sync.dma_start(out=stats_ar[:], in_=stats_out[0:OUT, :])
            trp0 = ps.tile([1, OUT], F32, tag="ps")
            nc.tensor.matmul(out=trp0[:], lhsT=stats_ar[:, 0:1],
                             rhs=ident[0:OUT, 0:OUT], start=True, stop=True)
            trp1 = ps.tile([1, OUT], F32, tag="ps")
            nc.tensor.matmul(out=trp1[:], lhsT=stats_ar[:, 1:2],
                             rhs=ident[0:OUT, 0:OUT], start=True, stop=True)
            mean = sb1.tile([1, OUT], F32, tag="mean")
            nc.vector.tensor_scalar(mean[:], trp0[:], 1.0 / nn, None, AluOp.mult)
            ev2 = sb1.tile([1, OUT], F32, tag="ev2")
            nc.vector.tensor_scalar(ev2[:], trp1[:], 1.0 / nn, None, AluOp.mult)
            m2 = sb1.tile([1, OUT], F32, tag="m2")
            nc.vector.tensor_tensor(out=m2[:], in0=mean[:], in1=mean[:],
                                    op=AluOp.mult)
            var = sb1.tile([1, OUT], F32, tag="var")
            nc.vector.tensor_tensor(out=var[:], in0=ev2[:], in1=m2[:],
                                    op=AluOp.subtract)
            nc.vector.tensor_scalar(var[:], var[:], EPS, None, AluOp.add)
            std = sb1.tile([1, OUT], F32, tag="std")
            nc.scalar.sqrt(std[:], var[:])
            rstd = sb1.tile([1, OUT], F32, tag="rstd")
            nc.vector.reciprocal(rstd[:], std[:])
            bng = sb1.tile([1, OUT], F32, tag="bng")
            bnb = sb1.tile([1, OUT], F32, tag="bnb")
            if last:
                nc.sync.dma_start(out=bng[:], in_=ins["bn_g_l"][:])
                nc.sync.dma_start(out=bnb[:], in_=ins["bn_b_l"][:])
            else:
                nc.sync.dma_start(out=bng[:], in_=ins["bn_g"][li])
                nc.sync.dma_start(out=bnb[:], in_=ins["bn_b"][li])
            sg = sb1.tile([1, OUT], F32, tag="sg")
            nc.vector.tensor_tensor(out=sg[:], in0=rstd[:], in1=bng[:],
                                    op=AluOp.mult)
            c0 = sb1.tile([1, OUT], F32, tag="c0")
            nc.vector.tensor_tensor(out=c0[:], in0=mean[:], in1=sg[:],
                                    op=AluOp.mult)
            crow = sb1.tile([1, OUT], F32, tag="crow")
            nc.vector.tensor_tensor(out=crow[:], in0=bnb[:], in1=c0[:],
                                    op=AluOp.subtract)
            reps = []
            for rsrc in (sg, crow):
                rp = ps.tile([128, OUT], F32, tag="ps")
                nc.tensor.matmul(out=rp[:], lhsT=onesrow[:], rhs=rsrc[:],
                                 start=True, stop=True)
                rt = sb1.tile([128, OUT], F32, tag=f"rep{len(reps)}")
                nc.scalar.copy(out=rt[:], in_=rp[:])
                reps.append(rt)

            def rep_b(rt):
                return (rt[:].rearrange("p (o c) -> p o c", o=1)
                        .broadcast_to([128, NG, OUT]))

            bn = sq  # reuse buffer
            aggv = agg[:].rearrange("p (g c) -> p g c", c=OUT)
            bnv = bn[:].rearrange("p (g c) -> p g c", c=OUT)
            nc.vector.tensor_tensor(out=bnv, in0=aggv, in1=rep_b(reps[0]),
                                    op=AluOp.mult)
            nc.vector.tensor_tensor(out=bnv, in0=bnv, in1=rep_b(reps[1]),
                                    op=AluOp.add)
            nc.vector.tensor_scalar(bn[:], bn[:], 0.0, None, AluOp.max)

            if last:
                nc.sync.dma_start(out=outs["out"][:], in_=bn[:])
            else:
                h_new = sb.tile([128, NG * HID], F32, tag="h")
                nc.vector.tensor_tensor(out=h_new[:], in0=bn[:], in1=h_cur[:],
                                        op=AluOp.add)
                h_cur = h_new
                push_table(h_cur[:])

    for _rep in range(NREPEAT):
        one_forward()

    stack.close()


# ---------------------------------------------------------------------------
# top-level entry
# ---------------------------------------------------------------------------

def _make_in_maps(g, weights):
    in_maps = []
    for c in range(NCORES):
        pc = g["per_core"][c]
        m = dict(weights)
        m["featT"] = g["featT"][c]
        m["ident"] = np.eye(g["hid"], dtype=np.float32)
        m["idx_ev"] = pc["idx_ev"]
        m["idx_od"] = pc["idx_od"]
        m["eq"] = pc["eq"]
        m["dr"] = pc["dr"]
        m["dc"] = pc["dc"]
        in_maps.append({k + "_d": v for k, v in m.items()})
    return in_maps


def _weights_dict(inputs, g):
    f32 = lambda x: np.ascontiguousarray(np.asarray(x, np.float32))
    bf16 = lambda x: np.ascontiguousarray(np.asarray(x, ml_dtypes.bfloat16))
    nhl, k, hid, ncls = g["nhl"], g["k"], g["hid"], g["ncls"]
    fc_wb = np.concatenate(
        [np.asarray(inputs["fc_w"], np.float32),
         np.asarray(inputs["fc_b"], np.float32).reshape(nhl, 1, k * hid)],
        axis=1)                                      # [nhl, 65, k*hid]
    fc_wb_l = np.concatenate(
        [np.asarray(inputs["fc_w_l"], np.float32),
         np.asarray(inputs["fc_b_l"], np.float32).reshape(1, k * ncls)],
        axis=0)                                      # [65, k*ncls]
    w = dict(
        emb_w=f32(inputs["emb_w"]),                  # [128, 64]
        emb_b=f32(inputs["emb_b"]).reshape(1, -1),
        fc_wb=f32(fc_wb),
        fc_wb_l=f32(fc_wb_l),
        mu=f32(inputs["mu"]).reshape(nhl, 1, -1),
        inv_sigma=f32(inputs["inv_sigma"]).reshape(nhl, 1, -1),
        pp_w=f32(inputs["pp_w"]).reshape(nhl, 1, -1),
        pp_b=f32(inputs["pp_b"]).reshape(nhl, 1, -1),
        bn_g=f32(inputs["bn_g"]).reshape(nhl, 1, -1),
        bn_b=f32(inputs["bn_b"]).reshape(nhl, 1, -1),
        mu_l=f32(inputs["mu_l"]).reshape(1, -1),
        inv_sigma_l=f32(inputs["inv_sigma_l"]).reshape(1, -1),
        pp_w_l=f32(inputs["pp_w_l"]).reshape(1, -1),
        pp_b_l=f32(inputs["pp_b_l"]).reshape(1, -1),
        bn_g_l=f32(inputs["bn_g_l"]).reshape(1, -1),
        bn_b_l=f32(inputs["bn_b_l"]).reshape(1, -1),
    )
    return w


def _build_featT(inputs, g):
    feat = np.asarray(inputs["feature"], np.float32)
    featT = []
    for c in range(NCORES):
        arr = np.zeros((g["in_dim"], g["npc"]), np.float32)
        nds = np.flatnonzero(g["core_of"] == c)
        arr[:, g["slot_of"][nds] % g["npc"]] = feat[nds].T
        featT.append(arr)
    g["featT"] = featT


def run_device(g, weights, trace=False):
    nc = bacc.Bacc("TRN2", target_bir_lowering=False, debug=False,
                   num_devices=NCORES, num_swdge_queues=4)
    ins_ap, outs_ap = {}, {}
    in_maps = _make_in_maps(g, weights)
    for name, arr in in_maps[0].items():
        t = nc.dram_tensor(name, list(arr.shape), mybir.dt.from_np(arr.dtype),
                           kind="ExternalInput")
        ins_ap[name[:-2]] = t.ap()
    out_t = nc.dram_tensor("out_d", [128, g["NG"] * g["ncls"]], F32,
                           kind="ExternalOutput")
    outs_ap["out"] = out_t.ap()

    with tile.TileContext(nc) as tc:
        build(tc, outs_ap, ins_ap, g)
    nc.compile()

    res = bass_utils.run_bass_kernel_spmd(
        nc, in_maps, core_ids=list(range(NCORES)), trace=trace)
    return res


def assemble_output(g, res):
    out = np.zeros((g["n"], g["ncls"]), np.float32)
    for c in range(NCORES):
        oc = res.results[c]["out_d"].reshape(128, g["NG"], g["ncls"])
        nds = np.flatnonzero(g["core_of"] == c)
        sl = g["slot_of"][nds] % g["npc"]
        out[nds] = oc[sl % 128, sl // 128, :]
    return out


def kernel(**inputs):
    g = preprocess(np.asarray(inputs["edge_index"]), GEOM_REAL)
    _build_featT(inputs, g)
    weights = _weights_dict(inputs, g)
    res = run_device(g, weights, trace=os.environ.get("MONET_TRACE", "0") == "1")
    out = assemble_output(g, res)
    kernel.last_exec_time_ns = getattr(res, "exec_time_ns", None)
    return out


# ---------------------------------------------------------------------------
# timed execution (repeated PJRT calls on a single compiled executable)
# ---------------------------------------------------------------------------

def run_device_timed(g, weights, n_iters=5):
    import time
    import jax
    from jax.sharding import Mesh, PartitionSpec
    from jax.experimental.shard_map import shard_map
    from concourse import bass2jax as b2j

    nc = bacc.Bacc("TRN2", target_bir_lowering=False, debug=False,
                   num_devices=NCORES, num_swdge_queues=4)
    ins_ap = {}
    in_maps = _make_in_maps(g, weights)
    for name, arr in in_maps[0].items():
        t = nc.dram_tensor(name, list(arr.shape), mybir.dt.from_np(arr.dtype),
                           kind="ExternalInput")
        ins_ap[name[:-2]] = t.ap()
    out_t = nc.dram_tensor("out_d", [128, g["NG"] * g["ncls"]], F32,
                           kind="ExternalOutput")
    outs_ap = {"out": out_t.ap()}
    with tile.TileContext(nc) as tc:
        build(tc, outs_ap, ins_ap, g)
    nc.compile()

    b2j.install_neuronx_cc_hook()
    partition_name = (nc.partition_id_tensor.name
                      if nc.partition_id_tensor else None)
    in_names, out_names, out_avals, zero_outs = [], [], [], []
    for alloc in nc.m.functions[0].allocations:
        if not isinstance(alloc, mybir.MemoryLocationSet):
            continue
        name = alloc.memorylocations[0].name
        if alloc.kind == "ExternalInput":
            if name != partition_name:
                in_names.append(name)
        elif alloc.kind == "ExternalOutput":
            dt = mybir.dt.np(alloc.dtype)
            out_avals.append(jax.core.ShapedArray(tuple(alloc.tensor_shape), dt))
            out_names.append(name)
            zero_outs.append(np.zeros(tuple(alloc.tensor_shape), dt))
    n_params = len(in_names)
    n_outs = len(out_names)
    in_names = in_names + out_names
    if partition_name is not None:
        in_names.append(partition_name)
    donate = tuple(range(n_params, n_params + n_outs))

    def _body(*args):
        operands = list(args)
        if partition_name is not None:
            operands.append(b2j.partition_id_tensor())
        outs = b2j._bass_exec_p.bind(
            *operands,
            out_avals=tuple(out_avals),
            in_names=tuple(in_names),
            out_names=tuple(out_names),
            lowering_input_output_aliases=(),
            sim_require_finite=True,
            sim_require_nnan=True,
            nc=nc,
        )
        return tuple(outs)

    devices = jax.devices()[:NCORES]
    mesh = Mesh(np.asarray(devices), ("core",))
    sharded = jax.jit(
        shard_map(_body, mesh=mesh,
                  in_specs=(PartitionSpec("core"),) * (n_params + n_outs),
                  out_specs=(PartitionSpec("core"),) * n_outs,
                  check_rep=False),
        donate_argnums=donate, keep_unused=True)
    per_core = [[np.asarray(m[nm]) for nm in in_names[:n_params]]
                for m in in_maps]
    concat_in = [np.concatenate([per_core[c][i] for c in range(NCORES)], 0)
                 for i in range(n_params)]
    concat_in = [jax.device_put(a) for a in concat_in]

    times = []
    out_arrs = None
    for it in range(n_iters):
        czeros = [np.zeros((NCORES * z.shape[0], *z.shape[1:]), z.dtype)
                  for z in zero_outs]
        t0 = time.perf_counter()
        out_arrs = sharded(*concat_in, *czeros)
        jax.block_until_ready(out_arrs)
        times.append(time.perf_counter() - t0)
    results = [
        {nm: np.asarray(out_arrs[i]).reshape(NCORES, *out_avals[i].shape)[c]
         for i, nm in enumerate(out_names)}
        for c in range(NCORES)
    ]

    class R:
        pass
    r = R()
    r.results = results
    r.exec_time_ns = int(min(times[1:]) * 1e9) if len(times) > 1 else None
    r.all_times = times
    return r


# revision 9
# speedup vs baseline: 49.0878x; 1.0481x over previous
"""MoNet (GMM graph conv) on Trainium2 — 8-core SPMD Bass/Tile kernel.

Sharding: dst-node slices per core (edge-parallel within core), with node
relabeling into per-core "slot space". Uniform SPMD program; per-core data.

v2 layout: gapless edge tiles. Per (chunk, window, class) a GLOBAL capacity
(max edge count over cores) reserves a contiguous position range; positions
pack into 128-wide gather tiles that may straddle window boundaries. A
(tile, window) pair is a "segment": the PE matmul for window w over a
boundary tile uses an s3 block whose other-window partitions are zero.
 - gather: dma_gather 256B rows (64 bf16 feats + ones col + pad) via two
   stride-2 table views (int16 idx range), indices sorted within each
   (window, class) run for HBM locality.
 - s3 = one-hot(dst slot) x gauss built by DVE from uint8 one-hot.
 - PSUM: 4 windows share one bank [65, 4, K*32]; one Act copy stages all 4
   to bf16 ust; fc matmuls in bf16.
 - BN stats via DVE group-reduce + 1-col PE matmuls + AllReduce; h pushed
   compact (64 cols bf16) through a Shared-output AllGather into the
   256B-row gather table (ones column initialized once).
"""
import sys, os
import numpy as np

if "/opt/trn_rl_repo" not in sys.path:
    sys.path.insert(0, "/opt/trn_rl_repo")

import ml_dtypes
from concourse import bass, bacc, mybir, tile
from concourse import bass_utils

AluOp = mybir.AluOpType
Act = mybir.ActivationFunctionType
F32 = mybir.dt.float32
BF16 = mybir.dt.bfloat16
I16 = mybir.dt.int16
U8 = mybir.dt.uint8

NCORES = 8
EPS = 1e-5

GEOM_REAL = dict(n=50000, e=800000, in_dim=128, hid=64, k=3, pdim=2,
                 ncls=16, nhl=3, W=208, wpchunk=16)


def derive(geom):
    g = dict(geom)
    g["npc"] = g["W"] * 32                # dst slots per core
    g["NG"] = g["npc"] // 128             # 128-slot groups per core
    g["NCH"] = g["W"] // g["wpchunk"]     # chunks per layer
    g["n_rows"] = NCORES * g["npc"]       # table rows
    assert g["n_rows"] // 2 <= 32767
    return g


# ---------------------------------------------------------------------------
# host preprocessing (pure integer/index manipulation)
# ---------------------------------------------------------------------------

def preprocess(edge_index, geom):
    g = derive(geom)
    n, W, npc = g["n"], g["W"], g["npc"]
    NCH, wpc = g["NCH"], g["wpchunk"]
    row = np.asarray(edge_index[0], np.int64)
    col = np.asarray(edge_index[1], np.int64)
    deg_r = np.bincount(row, minlength=n).astype(np.int64)
    deg_c = np.bincount(col, minlength=n).astype(np.int64)

    # 1) nodes -> cores (snake deal by in-degree for balanced edge counts)
    order = np.argsort(-deg_c, kind="stable")
    core_of = np.empty(n, np.int64)
    blk = np.arange(n) // NCORES
    pos = np.arange(n) % NCORES
    snake = np.where(blk % 2 == 0, pos, NCORES - 1 - pos)
    core_of[order] = snake

    # 2) class A (even table rows) = per-core top half by out-degree
    is_a = np.zeros(n, bool)
    for c in range(NCORES):
        nds = np.flatnonzero(core_of == c)
        half = min((len(nds) + 1) // 2, W * 16)
        topa = nds[np.argsort(-deg_r[nds], kind="stable")][:half]
        is_a[topa] = True

    src_a = is_a[row]
    in_ev = np.bincount(col[src_a], minlength=n).astype(np.int64)
    in_od = np.bincount(col[~src_a], minlength=n).astype(np.int64)

    # 3) per-core window packing (first-fit decreasing, loose caps only to
    #    balance chunks) then window relabel by total load so that per-rank
    #    counts align across cores (tight global caps).
    cap_ev, cap_od = 6 * 128, 4 * 128
    slot_of = np.full(n, -1, np.int64)
    wcnt_ev = np.zeros((NCORES, W), np.int64)
    wcnt_od = np.zeros((NCORES, W), np.int64)
    for c in range(NCORES):
        nds = np.flatnonzero(core_of == c)
        nds = nds[np.argsort(-(in_ev[nds] + in_od[nds]), kind="stable")]
        wev = np.zeros(W, np.int64); wod = np.zeros(W, np.int64)
        wna = np.zeros(W, np.int64); wnb = np.zeros(W, np.int64)
        wslot = np.full(n, -1, np.int64)
        for nd in nds:
            a = bool(is_a[nd])
            for w in range(W):
                if a and wna[w] >= 16: continue
                if (not a) and wnb[w] >= 16: continue
                if wev[w] + in_ev[nd] > cap_ev: continue
                if wod[w] + in_od[nd] > cap_od: continue
                if a:
                    j = 2 * wna[w]; wna[w] += 1
                else:
                    j = 2 * wnb[w] + 1; wnb[w] += 1
                wev[w] += in_ev[nd]; wod[w] += in_od[nd]
                wslot[nd] = w * 32 + j
                break
            else:
                raise RuntimeError(f"window packing failed (core {c})")
        # relabel windows: sort by (ev+od) load descending so rank k has the
        # k-th largest load on every core -> per-rank max over cores ~= mean
        perm = np.argsort(-(wev + wod), kind="stable")   # rank -> old w
        rank_of = np.empty(W, np.int64)
        # deal ranks round-robin over chunks: rank r -> window
        # (r % NCH)*wpc + r//NCH, so every chunk gets a balanced mix and
        # per-rank loads still align across cores
        rr = (np.arange(W) % NCH) * wpc + np.arange(W) // NCH
        rank_of[perm] = rr
        sel = wslot >= 0
        slot_of[sel] = (core_of[sel] * npc + rank_of[wslot[sel] // 32] * 32
                        + wslot[sel] % 32)
        wcnt_ev[c][rr] = wev[perm]
        wcnt_od[c][rr] = wod[perm]

    assert (slot_of >= 0).all()
    assert (slot_of[is_a] % 2 == 0).all() and (slot_of[~is_a] % 2 == 1).all()

    g.update(core_of=core_of, slot_of=slot_of, deg_r=deg_r, deg_c=deg_c)

    # 4) global segment template from per-(window, class) caps
    caps_ev = wcnt_ev.max(axis=0)        # [W]
    caps_od = wcnt_od.max(axis=0)
    EVN = np.zeros(NCH, np.int64)        # gather idx count per chunk (even)
    ODN = np.zeros(NCH, np.int64)
    NSEG = np.zeros(NCH, np.int64)
    seg_template = []                    # per ch: dict keyed (wl, cls, tile)->col
    base_ev = np.zeros(W, np.int64)      # position base of window's run
    base_od = np.zeros(W, np.int64)
    win_segs = []                        # per ch: list over wl of [(cls, tile, col)]
    for ch in range(NCH):
        ws = range(ch * wpc, (ch + 1) * wpc)
        segcol = {}
        wl_segs = [[] for _ in range(wpc)]
        for cls, caps, base_arr, tot in ((0, caps_ev, base_ev, None),
                                         (1, caps_od, base_od, None)):
            p0 = 0
            for wl, w in enumerate(ws):
                base_arr[w] = p0
                cap = int(caps[w])
                if cap > 0:
                    for t in range(p0 // 128, (p0 + cap - 1) // 128 + 1):
                        key = (wl, cls, t)
                        if key not in segcol:
                            segcol[key] = -1   # placeholder, ordered later
                        wl_segs[wl].append((cls, t))
                p0 += cap
            if cls == 0:
                EVN[ch] = -(-p0 // 128) * 128
            else:
                ODN[ch] = -(-p0 // 128) * 128
        # assign columns ordered by (wl, cls, tile)
        cols = 0
        for wl in range(wpc):
            newl = []
            for cls, t in wl_segs[wl]:
                segcol[(wl, cls, t)] = cols
                newl.append((cls, t, cols))
                cols += 1
            wl_segs[wl] = newl
        NSEG[ch] = cols
        seg_template.append(segcol)
        win_segs.append(wl_segs)

    SEG_CAP = int(NSEG.max())
    EVT_CAP = int((EVN // 128).max())
    ODT_CAP = int((ODN // 128).max())
    g.update(EVN=EVN.tolist(), ODN=ODN.tolist(), NSEG=NSEG.tolist(),
             SEG_CAP=SEG_CAP, EVT_CAP=EVT_CAP, ODT_CAP=ODT_CAP,
             win_segs=win_segs, SEGT=NCH * SEG_CAP)
    tot_slots = int(EVN.sum() + ODN.sum())
    g["gather_slots"] = tot_slots

    # 5) per-core arrays
    e_core = core_of[col]
    e_slot = slot_of[col] % npc
    e_w = e_slot // 32
    e_j = e_slot % 32
    e_view = (slot_of[row] // 2).astype(np.int64)   # stride-2 view index

    per_core = []
    for c in range(NCORES):
        idx_ev = np.zeros((NCH, 16, EVT_CAP * 8), np.int16)
        idx_od = np.zeros((NCH, 16, ODT_CAP * 8), np.int16)
        eq = np.zeros((NCH, 128, SEG_CAP * 32), np.uint8)
        dr = np.zeros((128, NCH * SEG_CAP), np.float32)
        dc = np.zeros((128, NCH * SEG_CAP), np.float32)

        sel = np.flatnonzero(e_core == c)
        ew, ej, ecls = e_w[sel], e_j[sel], src_a[sel]
        evi = e_view[sel]
        edr = deg_r[row[sel]].astype(np.float32)
        edc = deg_c[col[sel]].astype(np.float32)
        # group edges by (window, class); sort by view index inside a group
        okey = ew * 2 + (~ecls).astype(np.int64)
        eorder = np.lexsort((evi, okey))
        bnd = np.searchsorted(okey[eorder], np.arange(2 * W + 1))
        for w in range(W):
            ch, wl = divmod(w, wpc)
            segcol = seg_template[ch]
            for cls in (0, 1):
                kk = w * 2 + cls
                eids = eorder[bnd[kk]:bnd[kk + 1]]
                ne = len(eids)
                if ne == 0:
                    continue
                b = base_ev[w] if cls == 0 else base_od[w]
                q = np.arange(ne)
                pp = (b + q) % 128
                tt = (b + q) // 128
                scol = np.array([segcol[(wl, cls, int(t))] for t in tt])
                tgt = idx_ev if cls == 0 else idx_od
                tgt[ch][(b + q) % 16, (b + q) // 16] = evi[eids].astype(np.int16)
                eq[ch][pp, scol * 32 + ej[eids]] = 1
                dr[pp, ch * SEG_CAP + scol] = edr[eids]
                dc[pp, ch * SEG_CAP + scol] = edc[eids]
        idx_ev = np.tile(idx_ev, (1, 8, 1))
        idx_od = np.tile(idx_od, (1, 8, 1))
        per_core.append(dict(idx_ev=idx_ev, idx_od=idx_od, eq=eq, dr=dr, dc=dc))
    g["per_core"] = per_core
    return g


# ---------------------------------------------------------------------------
# device program
# ---------------------------------------------------------------------------

def build(tc, outs, ins, g):
    nc = tc.nc
    W, npc, NG = g["W"], g["npc"], g["NG"]
    NCH, wpc = g["NCH"], g["wpchunk"]
    HID, KK, NCLS, NHL = g["hid"], g["k"], g["ncls"], g["nhl"]
    n_rows = g["n_rows"]
    nn = g["n"]
    SEG_CAP, SEGT = g["SEG_CAP"], g["SEGT"]
    EVT_CAP, ODT_CAP = g["EVT_CAP"], g["ODT_CAP"]
    EVN, ODN, NSEG = g["EVN"], g["ODN"], g["NSEG"]
    win_segs = g["win_segs"]

    import contextlib
    stack = contextlib.ExitStack()
    sbc = stack.enter_context(tc.tile_pool(name="sbc", bufs=1))
    sb1 = stack.enter_context(tc.tile_pool(name="sb1", bufs=1))
    sb = stack.enter_context(tc.tile_pool(name="sb", bufs=2))
    ps = stack.enter_context(tc.tile_pool(name="ps", bufs=8, space="PSUM"))
    dram = stack.enter_context(tc.tile_pool(name="dram", bufs=1, space="DRAM"))

    # ---- constants / persistent state
    onesrow = sbc.tile([1, 128], F32); nc.vector.memset(onesrow[:], 1.0)
    onescol = sbc.tile([128, 1], F32); nc.vector.memset(onescol[:], 1.0)
    ident = sbc.tile([HID, HID], F32)
    nc.sync.dma_start(out=ident[:], in_=ins["ident"][:])
    stage = sbc.tile([128, NG, 128], BF16)
    nc.vector.memset(stage[:], 0.0)
    nc.vector.memset(stage[:, :, 64:65], 1.0)
    srcs = sbc.tile([128, SEGT], F32)
    dsts = sbc.tile([128, SEGT], F32)
    gauss = sbc.tile([128, KK, SEGT], BF16)

    table = dram.tile([n_rows, 128], BF16)
    stage_d = dram.tile([npc, 128], BF16)
    stats_in = dram.tile([HID, 2], F32)
    stats_out = dram.tile([HID, 2], F32)

    zz = sbc.tile([HID, 2], F32)
    nc.vector.memset(zz[:], 0.0)
    nc.sync.dma_start(out=stats_in[:], in_=zz[:])
    nc.sync.dma_start(out=stats_out[:], in_=zz[:])

    tbl_ev = table[:].rearrange("(m two) c -> m (two c)", two=2)[:, 0:128]
    tbl_od = table[:].rearrange("(m two) c -> m (two c)", two=2)[:, 128:256]

    # ---- prologue: pseudo coords
    with tc.tile_pool(name="pro", bufs=1) as pro:
        drt = pro.tile([128, SEGT], F32)
        nc.sync.dma_start(out=drt[:], in_=ins["dr"][:])
        dct = pro.tile([128, SEGT], F32)
        nc.sync.dma_start(out=dct[:], in_=ins["dc"][:])
        t0 = pro.tile([128, SEGT], F32)
        for dsrc, dout in ((drt, srcs), (dct, dsts)):
            nc.vector.tensor_scalar(t0[:], dsrc[:], 1.0, None, AluOp.add)
            nc.scalar.sqrt(t0[:], t0[:])
            nc.vector.reciprocal(dout[:], t0[:])

    NO_CC = os.environ.get("MONET_NO_CC", "0") == "1"
    NHID_RUN = int(os.environ.get("MONET_NLAYERS", str(NHL)))
    NREPEAT = int(os.environ.get("MONET_REPEAT", "1"))

    def push_table(h_flat):
        # h_flat [128, NG*64] f32 -> bf16 stage (+ones col) -> AllGather
        nc.vector.tensor_copy(
            out=stage[:, :, 0:64],
            in_=h_flat.rearrange("p (g c) -> p g c", c=64))
        nc.sync.dma_start(
            out=stage_d[:].rearrange("(gp p) c -> p gp c", p=128),
            in_=stage[:])
        if NO_CC:
            nc.sync.dma_start(out=table[0:npc, :], in_=stage_d[:])
            return
        nc.gpsimd.collective_compute(
            "AllGather", AluOp.bypass, replica_groups=[list(range(NCORES))],
            ins=[stage_d[:].opt()], outs=[table[:].opt()])

    def one_forward():
        # ---- embed: h0 = featT.T @ emb_w + emb_b
        h_cur = sb.tile([128, NG * HID], F32, tag="h")
        with tc.tile_pool(name="emb", bufs=2) as emb:
            embw = emb.tile([128, HID], F32, tag="embw")
            nc.sync.dma_start(out=embw[:], in_=ins["emb_w"][:])
            ebrow = emb.tile([1, HID], F32, tag="ebrow")
            nc.sync.dma_start(out=ebrow[:], in_=ins["emb_b"][:])
            for gi in range(NG):
                ft = emb.tile([128, 128], F32, tag="ft")
                nc.sync.dma_start(out=ft[:],
                                  in_=ins["featT"][:, gi * 128:(gi + 1) * 128])
                ep = ps.tile([128, HID], F32, tag="ps")
                nc.tensor.matmul(out=ep[:], lhsT=ft[:],
                                 rhs=embw[:], start=True, stop=True)
                nc.scalar.copy(out=h_cur[:, gi * HID:(gi + 1) * HID], in_=ep[:])
            ebp = ps.tile([128, HID], F32, tag="ps")
            nc.tensor.matmul(out=ebp[:], lhsT=onesrow[:], rhs=ebrow[:],
                             start=True, stop=True)
            ebrep = emb.tile([128, HID], F32)
            nc.scalar.copy(out=ebrep[:], in_=ebp[:])
            nc.vector.tensor_tensor(
                out=h_cur[:], in0=h_cur[:],
                in1=ebrep[:].rearrange("p (o c) -> p o c", o=1)
                    .broadcast_to([128, NG, HID]),
                op=AluOp.add)
        push_table(h_cur[:])

        # ---- layers
        for li in list(range(NHID_RUN)) + [NHL]:
            last = li == NHL
            OUT = NCLS if last else HID

            # scalars row: [w00 w01 w10 w11 b0 b1 | mu k*2+d | isg k*2+d]
            scal_row = sb1.tile([1, 32], F32, tag="scalrow")
            nc.vector.memset(scal_row[:], 0.0)
            if last:
                nc.sync.dma_start(out=scal_row[:, 0:4], in_=ins["pp_w_l"][:])
                nc.sync.dma_start(out=scal_row[:, 4:6], in_=ins["pp_b_l"][:])
                nc.sync.dma_start(out=scal_row[:, 6:6 + 2 * KK], in_=ins["mu_l"][:])
                nc.sync.dma_start(out=scal_row[:, 18:18 + 2 * KK],
                                  in_=ins["inv_sigma_l"][:])
            else:
                nc.sync.dma_start(out=scal_row[:, 0:4], in_=ins["pp_w"][li])
                nc.sync.dma_start(out=scal_row[:, 4:6], in_=ins["pp_b"][li])
                nc.sync.dma_start(out=scal_row[:, 6:6 + 2 * KK], in_=ins["mu"][li])
                nc.sync.dma_start(out=scal_row[:, 18:18 + 2 * KK],
                                  in_=ins["inv_sigma"][li])
            scp = ps.tile([128, 32], F32, tag="ps")
            nc.tensor.matmul(out=scp[:], lhsT=onesrow[:], rhs=scal_row[:],
                             start=True, stop=True)
            scal = sb1.tile([128, 32], F32, tag="scal")
            nc.scalar.copy(out=scal[:], in_=scp[:])

            def sc(j):
                return scal[:, j:j + 1]

            # gauss[k] = exp(-0.5*(((ps0-mu_k0)*is_k0)^2+((ps1-mu_k1)*is_k1)^2))
            ps0 = sb1.tile([128, SEGT], F32, tag="ps0")
            ps1 = sb1.tile([128, SEGT], F32, tag="ps1")
            ta = sb1.tile([128, SEGT], F32, tag="ta")
            tb = sb1.tile([128, SEGT], F32, tag="tb")
            for (pst, wA, wB, bB) in ((ps0, 0, 2, 4), (ps1, 1, 3, 5)):
                nc.vector.tensor_scalar(ta[:], srcs[:], sc(wA), None, AluOp.mult)
                nc.vector.tensor_scalar(tb[:], dsts[:], sc(wB), None, AluOp.mult)
                nc.vector.tensor_tensor(out=ta[:], in0=ta[:], in1=tb[:],
                                        op=AluOp.add)
                nc.scalar.activation(pst[:], ta[:], Act.Tanh, bias=sc(bB),
                                     scale=1.0)
            for k in range(KK):
                nc.vector.tensor_scalar(ta[:], ps0[:], sc(6 + 2 * k),
                                        sc(18 + 2 * k),
                                        AluOp.subtract, AluOp.mult)
                nc.vector.tensor_scalar(tb[:], ps1[:], sc(7 + 2 * k),
                                        sc(19 + 2 * k),
                                        AluOp.subtract, AluOp.mult)
                nc.scalar.square(ta[:], ta[:])
                nc.scalar.square(tb[:], tb[:])
                nc.vector.tensor_tensor(out=ta[:], in0=ta[:], in1=tb[:],
                                        op=AluOp.add)
                nc.scalar.activation(gauss[:, k, :], ta[:], Act.Exp,
                                     bias=0.0, scale=-0.5)

            # dense weights [65, K*OUT] bf16 (stacked [w; b] on host)
            fcwb = sb1.tile([65, KK * OUT], F32, tag="fcwb")
            if last:
                nc.sync.dma_start(out=fcwb[:], in_=ins["fc_wb_l"][:])
            else:
                nc.sync.dma_start(out=fcwb[:], in_=ins["fc_wb"][li])

            agg = sb1.tile([128, NG * OUT], F32, tag="aggsb")

            # ---- edge pipeline
            for ch in range(NCH):
                nev_t, nod_t = EVN[ch] // 128, ODN[ch] // 128
                nseg = NSEG[ch]
                FULLG = os.environ.get("MONET_FULLG", "0") == "1"
                evn = EVT_CAP * 128 if FULLG else EVN[ch]
                odn = ODT_CAP * 128 if FULLG else ODN[ch]
                iev = sb.tile([128, EVT_CAP * 8], I16, tag="iev")
                nc.sync.dma_start(out=iev[:, 0:evn // 16],
                                  in_=ins["idx_ev"][ch][:, 0:evn // 16])
                iod = sb.tile([128, ODT_CAP * 8], I16, tag="iod")
                nc.sync.dma_start(out=iod[:, 0:odn // 16],
                                  in_=ins["idx_od"][ch][:, 0:odn // 16])
                eqt = sb.tile([128, SEG_CAP, 32], U8, tag="eq")
                nc.sync.dma_start(
                    out=eqt[:].rearrange("p s j -> p (s j)")[:, 0:nseg * 32],
                    in_=ins["eq"][ch][:, 0:nseg * 32])
                hg_lo = sb.tile([128, EVT_CAP, 128], BF16, tag="hglo")
                hg_hi = sb.tile([128, ODT_CAP, 128], BF16, tag="hghi")
                if os.environ.get("MONET_NO_GATHER", "0") == "1":
                    nc.vector.memset(hg_lo[:], 0.5)
                    nc.vector.memset(hg_hi[:], 0.5)
                else:
                    # split each class gather in half across queues: 4-way
                    # desc-gen parallelism within a single chunk (matters at
                    # layer starts when no second chunk is in flight yet)
                    calls = []
                    e1 = ((evn // 128 + 1) // 2) * 128
                    calls.append((hg_lo, tbl_ev, iev, 0, e1))
                    if evn > e1:
                        calls.append((hg_lo, tbl_ev, iev, e1, evn))
                    o1 = ((odn // 128 + 1) // 2) * 128
                    calls.append((hg_hi, tbl_od, iod, 0, o1))
                    if odn > o1:
                        calls.append((hg_hi, tbl_od, iod, o1, odn))
                    for qi, (hg, tbl, idxs, p0, p1) in enumerate(calls):
                        nc.gpsimd.dma_gather(
                            out_ap=hg[:, p0 // 128:p1 // 128, :], in_ap=tbl,
                            idxs_ap=idxs[:, p0 // 16:p1 // 16],
                            num_idxs=p1 - p0, num_idxs_reg=p1 - p0,
                            elem_size=128, elem_step=256, single_packet=False,
                            queue_num=(ch + qi) % 4)
                s3 = sb.tile([128, KK, SEG_CAP, 32], BF16, tag="s3")
                for k in range(KK):
                    nc.vector.tensor_tensor(
                        out=s3[:, k, 0:nseg, :], in0=eqt[:, 0:nseg, :],
                        in1=gauss[:, k, ch * SEG_CAP:ch * SEG_CAP + nseg]
                            .rearrange("p (s o) -> p s o", o=1)
                            .broadcast_to([128, nseg, 32]),
                        op=AluOp.mult)
                OLDPSUM = os.environ.get("MONET_OLDPSUM", "0") == "1"
                for wl in range(wpc):
                    sub = wl % 4
                    if OLDPSUM:
                        win = ps.tile([65, KK * 32], F32, tag="ps")
                    elif sub == 0:
                        winq = ps.tile([65, 4, KK * 32], F32, tag="ps")
                    segs = win_segs[ch][wl]
                    if not segs:
                        nc.vector.memset(
                            (win[:] if OLDPSUM else winq[:, sub, :]), 0.0)
                    for si, (cls, t, scol) in enumerate(segs):
                        lhs = (hg_lo if cls == 0 else hg_hi)[:, t, 0:65]
                        nc.tensor.matmul(
                            out=(win[:] if OLDPSUM else winq[:, sub, :]),
                            lhsT=lhs,
                            rhs=s3[:, :, scol, :],
                            start=(si == 0), stop=(si == len(segs) - 1))
                    if OLDPSUM:
                        if sub == 0:
                            ust = sb.tile([65, KK, 4, 32], F32, tag="ust")
                        nc.scalar.copy(
                            out=ust[:, :, sub, :],
                            in_=win[:].rearrange("u (k j) -> u k j", j=32))
                    if sub == 3:
                        gi = (ch * wpc + wl) // 4
                        if not OLDPSUM:
                            ust = sb.tile([65, KK, 4, 32], F32, tag="ust")
                            nc.vector.tensor_copy(
                                out=ust[:].rearrange("u k s j -> u s k j"),
                                in_=winq[:].rearrange("u s (k j) -> u s k j",
                                                      j=32))
                        ap_ = ps.tile([128, OUT], F32, tag="ps")
                        for k in range(KK):
                            lhsu = ust[:, k].rearrange("u s j -> u (s j)")
                            nc.tensor.matmul(
                                out=ap_[:], lhsT=lhsu,
                                rhs=fcwb[:, k * OUT:(k + 1) * OUT],
                                start=(k == 0), stop=(k == KK - 1))
                        nc.scalar.copy(out=agg[:, gi * OUT:(gi + 1) * OUT],
                                       in_=ap_[:])

            # ---- BN stats: group-reduce on DVE, cross-partition via 1-col PE
            sq = sb1.tile([128, NG * OUT], F32, tag="sq")
            nc.scalar.square(sq[:], agg[:])
            aggr = sb1.tile([128, OUT], F32, tag="aggr")
            nc.vector.tensor_reduce(
                out=aggr[:], in_=agg[:].rearrange("p (g c) -> p c g", c=OUT),
                axis=mybir.AxisListType.X, op=AluOp.add)
            sqr = sb1.tile([128, OUT], F32, tag="sqr")
            nc.vector.tensor_reduce(
                out=sqr[:], in_=sq[:].rearrange("p (g c) -> p c g", c=OUT),
                axis=mybir.AxisListType.X, op=AluOp.add)
            sump = ps.tile([OUT, 1], F32, tag="ps")
            nc.tensor.matmul(out=sump[:], lhsT=aggr[:], rhs=onescol[:],
                             start=True, stop=True)
            sqp = ps.tile([OUT, 1], F32, tag="ps")
            nc.tensor.matmul(out=sqp[:], lhsT=sqr[:], rhs=onescol[:],
                             start=True, stop=True)
            stats = sb1.tile([OUT, 2], F32, tag="stats")
            nc.scalar.copy(out=stats[:, 0:1], in_=sump[:])
            nc.scalar.copy(out=stats[:, 1:2], in_=sqp[:])
            nc.sync.dma_start(out=stats_in[0:OUT, :], in_=stats[:])
            if NO_CC:
                nc.sync.dma_start(out=stats_out[0:OUT, :], in_=stats_in[0:OUT, :])
            else:
                nc.gpsimd.collective_compute(
                    "AllReduce", AluOp.add,
                    replica_groups=[list(range(NCORES))],
                    ins=[stats_in[:].opt()], outs=[stats_out[:].opt()])
            stats_ar = sb1.tile([OUT, 2], F32, tag="statsar")
            nc.sync.dma_start(out=stats_ar[:], in_=stats_out[0:OUT, :])
            trp0 = ps.tile([1, OUT], F32, tag="ps")
            nc.tensor.matmul(out=trp0[:], lhsT=stats_ar[:, 0:1],
                             rhs=ident[0:OUT, 0:OUT], start=True, stop=True)
            trp1 = ps.tile([1, OUT], F32, tag="ps")
            nc.tensor.matmul(out=trp1[:], lhsT=stats_ar[:, 1:2],
                             rhs=ident[0:OUT, 0:OUT], start=True, stop=True)
            mean = sb1.tile([1, OUT], F32, tag="mean")
            nc.vector.tensor_scalar(mean[:], trp0[:], 1.0 / nn, None, AluOp.mult)
            ev2 = sb1.tile([1, OUT], F32, tag="ev2")
            nc.vector.tensor_scalar(ev2[:], trp1[:], 1.0 / nn, None, AluOp.mult)
            m2 = sb1.tile([1, OUT], F32, tag="m2")
            nc.vector.tensor_tensor(out=m2[:], in0=mean[:], in1=mean[:],
                                    op=AluOp.mult)
            var = sb1.tile([1, OUT], F32, tag="var")
            nc.vector.tensor_tensor(out=var[:], in0=ev2[:], in1=m2[:],
                                    op=AluOp.subtract)
            nc.vector.tensor_scalar(var[:], var[:], EPS, None, AluOp.add)
            std = sb1.tile([1, OUT], F32, tag="std")
            nc.scalar.sqrt(std[:], var[:])
            rstd = sb1.tile([1, OUT], F32, tag="rstd")
            nc.vector.reciprocal(rstd[:], std[:])
            bng = sb1.tile([1, OUT], F32, tag="bng")
            bnb = sb1.tile([1, OUT], F32, tag="bnb")
            if last:
                nc.sync.dma_start(out=bng[:], in_=ins["bn_g_l"][:])
                nc.sync.dma_start(out=bnb[:], in_=ins["bn_b_l"][:])
            else:
                nc.sync.dma_start(out=bng[:], in_=ins["bn_g"][li])
                nc.sync.dma_start(out=bnb[:], in_=ins["bn_b"][li])
            sg = sb1.tile([1, OUT], F32, tag="sg")
            nc.vector.tensor_tensor(out=sg[:], in0=rstd[:], in1=bng[:],
                                    op=AluOp.mult)
            c0 = sb1.tile([1, OUT], F32, tag="c0")
            nc.vector.tensor_tensor(out=c0[:], in0=mean[:], in1=sg[:],
                                    op=AluOp.mult)
            crow = sb1.tile([1, OUT], F32, tag="crow")
            nc.vector.tensor_tensor(out=crow[:], in0=bnb[:], in1=c0[:],
                                    op=AluOp.subtract)
            reps = []
            for rsrc in (sg, crow):
                rp = ps.tile([128, OUT], F32, tag="ps")
                nc.tensor.matmul(out=rp[:], lhsT=onesrow[:], rhs=rsrc[:],
                                 start=True, stop=True)
                rt = sb1.tile([128, OUT], F32, tag=f"rep{len(reps)}")
                nc.scalar.copy(out=rt[:], in_=rp[:])
                reps.append(rt)

            def rep_b(rt):
                return (rt[:].rearrange("p (o c) -> p o c", o=1)
                        .broadcast_to([128, NG, OUT]))

            bn = sq  # reuse buffer
            aggv = agg[:].rearrange("p (g c) -> p g c", c=OUT)
            bnv = bn[:].rearrange("p (g c) -> p g c", c=OUT)
            nc.vector.tensor_tensor(out=bnv, in0=aggv, in1=rep_b(reps[0]),
                                    op=AluOp.mult)
            nc.vector.tensor_tensor(out=bnv, in0=bnv, in1=rep_b(reps[1]),
                                    op=AluOp.add)
            nc.vector.tensor_scalar(bn[:], bn[:], 0.0, None, AluOp.max)

            if last:
                nc.sync.dma_start(out=outs["out"][:], in_=bn[:])
            else:
                h_new = sb.tile([128, NG * HID], F32, tag="h")
                nc.vector.tensor_tensor(out=h_new[:], in0=bn[:], in1=h_cur[:],
                                        op=AluOp.add)
                h_cur = h_new
                push_table(h_cur[:])

    for _rep in range(NREPEAT):
        one_forward()

    stack.close()


# ---------------------------------------------------------------------------
# top-level entry
# ---------------------------------------------------------------------------

def _make_in_maps(g, weights):
    in_maps = []
    for c in range(NCORES):
        pc = g["per_core"][c]
        m = dict(weights)
        m["featT"] = g["featT"][c]
        m["ident"] = np.eye(g["hid"], dtype=np.float32)
        m["idx_ev"] = pc["idx_ev"]
        m["idx_od"] = pc["idx_od"]
        m["eq"] = pc["eq"]
        m["dr"] = pc["dr"]
        m["dc"] = pc["dc"]
        in_maps.append({k + "_d": v for k, v in m.items()})
    return in_maps


def _weights_dict(inputs, g):
    f32 = lambda x: np.ascontiguousarray(np.asarray(x, np.float32))
    bf16 = lambda x: np.ascontiguousarray(np.asarray(x, ml_dtypes.bfloat16))
    nhl, k, hid, ncls = g["nhl"], g["k"], g["hid"], g["ncls"]
    fc_wb = np.concatenate(
        [np.asarray(inputs["fc_w"], np.float32),
         np.asarray(inputs["fc_b"], np.float32).reshape(nhl, 1, k * hid)],
        axis=1)                                      # [nhl, 65, k*hid]
    fc_wb_l = np.concatenate(
        [np.asarray(inputs["fc_w_l"], np.float32),
         np.asarray(inputs["fc_b_l"], np.float32).reshape(1, k * ncls)],
        axis=0)                                      # [65, k*ncls]
    w = dict(
        emb_w=f32(inputs["emb_w"]),                  # [128, 64]
        emb_b=f32(inputs["emb_b"]).reshape(1, -1),
        fc_wb=f32(fc_wb),
        fc_wb_l=f32(fc_wb_l),
        mu=f32(inputs["mu"]).reshape(nhl, 1, -1),
        inv_sigma=f32(inputs["inv_sigma"]).reshape(nhl, 1, -1),
        pp_w=f32(inputs["pp_w"]).reshape(nhl, 1, -1),
        pp_b=f32(inputs["pp_b"]).reshape(nhl, 1, -1),
        bn_g=f32(inputs["bn_g"]).reshape(nhl, 1, -1),
        bn_b=f32(inputs["bn_b"]).reshape(nhl, 1, -1),
        mu_l=f32(inputs["mu_l"]).reshape(1, -1),
        inv_sigma_l=f32(inputs["inv_sigma_l"]).reshape(1, -1),
        pp_w_l=f32(inputs["pp_w_l"]).reshape(1, -1),
        pp_b_l=f32(inputs["pp_b_l"]).reshape(1, -1),
        bn_g_l=f32(inputs["bn_g_l"]).reshape(1, -1),
        bn_b_l=f32(inputs["bn_b_l"]).reshape(1, -1),
    )
    return w


def _build_featT(inputs, g):
    feat = np.asarray(inputs["feature"], np.float32)
    featT = []
    for c in range(NCORES):
        arr = np.zeros((g["in_dim"], g["npc"]), np.float32)
        nds = np.flatnonzero(g["core_of"] == c)
        arr[:, g["slot_of"][nds] % g["npc"]] = feat[nds].T
        featT.append(arr)
    g["featT"] = featT


def run_device(g, weights, trace=False):
    nc = bacc.Bacc("TRN2", target_bir_lowering=False, debug=False,
                   num_devices=NCORES, num_swdge_queues=4)
    ins_ap, outs_ap = {}, {}
    in_maps = _make_in_maps(g, weights)
    for name, arr in in_maps[0].items():
        t = nc.dram_tensor(name, list(arr.shape), mybir.dt.from_np(arr.dtype),
                           kind="ExternalInput")
        ins_ap[name[:-2]] = t.ap()
    out_t = nc.dram_tensor("out_d", [128, g["NG"] * g["ncls"]], F32,
                           kind="ExternalOutput")
    outs_ap["out"] = out_t.ap()

    with tile.TileContext(nc) as tc:
        build(tc, outs_ap, ins_ap, g)
    nc.compile()

    res = bass_utils.run_bass_kernel_spmd(
        nc, in_maps, core_ids=list(range(NCORES)), trace=trace)
    return res


def assemble_output(g, res):
    out = np.zeros((g["n"], g["ncls"]), np.float32)
    for c in range(NCORES):
        oc = res.results[c]["out_d"].reshape(128, g["NG"], g["ncls"])
        nds = np.flatnonzero(g["core_of"] == c)
        sl = g["slot_of"][nds] % g["npc"]
        out[nds] = oc[sl % 128, sl // 128, :]
    return out


def kernel(**inputs):
    g = preprocess(np.asarray(inputs["edge_index"]), GEOM_REAL)
    _build_featT(inputs, g)
    weights = _weights_dict(inputs, g)
    res = run_device(g, weights, trace=os.environ.get("MONET_TRACE", "0") == "1")
    out = assemble_output(g, res)
    kernel.last_exec_time_ns = getattr(res, "exec_time_ns", None)
    return out


# ---------------------------------------------------------------------------
# timed execution (repeated PJRT calls on a single compiled executable)
# ---------------------------------------------------------------------------

def run_device_timed(g, weights, n_iters=5):
    import time
    import jax
    from jax.sharding import Mesh, PartitionSpec
    from jax.experimental.shard_map import shard_map
    from concourse import bass2jax as b2j

    nc = bacc.Bacc("TRN2", target_bir_lowering=False, debug=False,
                   num_devices=NCORES, num_swdge_queues=4)
    ins_ap = {}
    in_maps = _make_in_maps(g, weights)
    for name, arr in in_maps[0].items():
        t = nc.dram_tensor(name, list(arr.shape), mybir.dt.from_np(arr.dtype),
                           kind="ExternalInput")
        ins_ap[name[:-2]] = t.ap()
    out_t = nc.dram_tensor("out_d", [128, g["NG"] * g["ncls"]], F32,
                           kind="ExternalOutput")
    outs_ap = {"out": out_t.ap()}
    with tile.TileContext(nc) as tc:
        build(tc, outs_ap, ins_ap, g)
    nc.compile()

    b2j.install_neuronx_cc_hook()
    partition_name = (nc.partition_id_tensor.name
                      if nc.partition_id_tensor else None)
    in_names, out_names, out_avals, zero_outs = [], [], [], []
    for alloc in nc.m.functions[0].allocations:
        if not isinstance(alloc, mybir.MemoryLocationSet):
            continue
        name = alloc.memorylocations[0].name
        if alloc.kind == "ExternalInput":
            if name != partition_name:
                in_names.append(name)
        elif alloc.kind == "ExternalOutput":
            dt = mybir.dt.np(alloc.dtype)
            out_avals.append(jax.core.ShapedArray(tuple(alloc.tensor_shape), dt))
            out_names.append(name)
            zero_outs.append(np.zeros(tuple(alloc.tensor_shape), dt))
    n_params = len(in_names)
    n_outs = len(out_names)
    in_names = in_names + out_names
    if partition_name is not None:
        in_names.append(partition_name)
    donate = tuple(range(n_params, n_params + n_outs))

    def _body(*args):
        operands = list(args)
        if partition_name is not None:
            operands.append(b2j.partition_id_tensor())
        outs = b2j._bass_exec_p.bind(
            *operands,
            out_avals=tuple(out_avals),
            in_names=tuple(in_names),
            out_names=tuple(out_names),
            lowering_input_output_aliases=(),
            sim_require_finite=True,
            sim_require_nnan=True,
            nc=nc,
        )
        return tuple(outs)

    devices = jax.devices()[:NCORES]
    mesh = Mesh(np.asarray(devices), ("core",))
    sharded = jax.jit(
        shard_map(_body, mesh=mesh,
                  in_specs=(PartitionSpec("core"),) * (n_params + n_outs),
                  out_specs=(PartitionSpec("core"),) * n_outs,
                  check_rep=False),
        donate_argnums=donate, keep_unused=True)
    per_core = [[np.asarray(m[nm]) for nm in in_names[:n_params]]
                for m in in_maps]
    concat_in = [np.concatenate([per_core[c][i] for c in range(NCORES)], 0)
                 for i in range(n_params)]
    concat_in = [jax.device_put(a) for a in concat_in]

    times = []
    out_arrs = None
    for it in range(n_iters):
        czeros = [np.zeros((NCORES * z.shape[0], *z.shape[1:]), z.dtype)
                  for z in zero_outs]
        t0 = time.perf_counter()
        out_arrs = sharded(*concat_in, *czeros)
        jax.block_until_ready(out_arrs)
        times.append(time.perf_counter() - t0)
    results = [
        {nm: np.asarray(out_arrs[i]).reshape(NCORES, *out_avals[i].shape)[c]
         for i, nm in enumerate(out_names)}
        for c in range(NCORES)
    ]

    class R:
        pass
    r = R()
    r.results = results
    r.exec_time_ns = int(min(times[1:]) * 1e9) if len(times) > 1 else None
    r.all_times = times
    return r
